# revision 22
# baseline (speedup 1.0000x reference)
"""Trainium2 distributed Bass kernel for AdaptiveGatedAttentionFusion.

Sharding: each of the 8 cores owns (batch b = core//2, half hf = core%2 of the
1024 spatial positions) -> 512 tokens per core. All weights replicated.
Activations channel-major [C, pos] in SBUF.

v2: the 24 heavy QKV/O 1024x1024 GEMMs run in fp8-e4m3 DoubleRow mode
(256-deep contraction per matmul, ~1.7x TensorE streaming) with fp32 PSUM
accumulation and data-driven descale factors.  Normalized features are
provably bounded (per-token L2 norm == 1), so fixed fp8 scales cannot
overflow.  Projections / fusion / scores / attn@V stay bf16 (fp8 there blows
the error budget - measured by simulation).

Cross-core communication (pairwise, cores (2b, 2b+1) share batch b):
  - one AllReduce per attention block for the head_dim x head_dim score
    matrices (spatial contraction split across the pair), overlapped with
    the next block's QKV projections.
  - one tiny AllReduce per modality for the SE global-average-pool sums.
"""

from contextlib import ExitStack

import numpy as np
import ml_dtypes

import concourse.bacc as bacc
import concourse.bass as bass
import concourse.mybir as mybir
import concourse.tile as tile
from concourse.bass_utils import run_bass_kernel_spmd

F32 = mybir.dt.float32
BF16 = mybir.dt.bfloat16
FP8 = mybir.dt.float8e4
AF = mybir.ActivationFunctionType
ALU = mybir.AluOpType
DR = mybir.MatmulPerfMode.DoubleRow

NCORES = 8
B = 4
HW = 1024
NT = 512                    # tokens per core (half the positions of one batch)
BC = 1024
HEADS = 8
HD = 128
SEH = 64                    # BC / reduction(16)
TOTAL = 3 * BC
SCALE = float(HD) ** -0.5
P = 128
ASCALE = 16.0               # fp8 activation scale (|act| <= ~4 everywhere)

RGB_C, DEPTH_C, LIDAR_C = 512, 256, 64
# attention block wiring: (q_modality, kv_modality); 0=rgb 1=depth 2=lidar
BLOCKS = [(0, 1), (0, 2), (1, 0), (1, 2), (2, 0), (2, 1)]
# pairwise replica groups: cores (2b, 2b+1) hold the two halves of batch b
RG_PAIR = [[0, 1], [2, 3], [4, 5], [6, 7]]

bf16 = ml_dtypes.bfloat16
e4m3 = ml_dtypes.float8_e4m3   # TRN FP8_EXP4-compatible (bias 7, max 240)

_compiled = None
LAST_RESULTS = None       # BassKernelResults of last run (for test harness)
TRACE = False


def _declare_params(nc):
    p = {}

    def inp(name, shape, dt):
        p[name] = nc.dram_tensor(name, list(shape), dt, kind="ExternalInput").ap()

    # per-core activation shard, channel-major [C, NT]
    inp("x_rgb", (RGB_C, NT), BF16)
    inp("x_depth", (DEPTH_C, NT), BF16)
    inp("x_lidar", (LIDAR_C, NT), BF16)
    # projection weights (transposed: [C_in, C_out]) bf16
    inp("w_rgb", (RGB_C, BC), BF16)
    inp("w_depth", (DEPTH_C, BC), BF16)
    inp("w_lidar", (LIDAR_C, BC), BF16)
    # attention weights: fp8 DoubleRow slabs, SBUF image layout
    # slab[p, ktp*2048 + g*1024 + n] = (w.T)[ktp*256 + g*128 + p, n] * s_w
    inp("w8_q", (6, P, 8192), FP8)
    inp("w8_k", (6, P, 8192), FP8)
    inp("w8_v", (6, P, 8192), FP8)
    inp("w8_o", (6, P, 8192), FP8)
    # q/k biases pre-broadcast to all 128 partitions (row 2i = q_b[i], 2i+1 = k_b[i])
    inp("qk_bias", (12, P, BC), BF16)
    # per-partition bias columns, f32, col j = channel tile j
    inp("bias_proj", (P, 24), F32)     # rgb 0:8, depth 8:16, lidar 16:24
    inp("bias_v", (P, 48), F32)        # block i cols 8i:8i+8
    inp("bias_o", (P, 24), F32)        # modality m cols 8m:8m+8 (pair-summed)
    # fp8 descale factors (per-partition broadcast columns):
    # cols 0-5 q[i], 6-11 k[i], 12-17 v[i], 18-20 o[m], col 21 row0 = var bias
    inp("desc", (P, 24), F32)
    # SE
    inp("w_se1", (3, BC, SEH), BF16)   # (se_w1/1024).T per modality
    inp("b_se1", (SEH, 3), F32)
    inp("w_se2", (3, SEH, BC), BF16)   # se_w2.T
    inp("b_se2", (P, 24), F32)         # modality m cols 8m:8m+8
    # gate
    inp("w_gate", (TOTAL, 3), BF16)    # gate_w[:, :3072].T
    inp("w_gate_x", (1, 9), BF16)      # seg m = gate_w[:, 3072+m] (var, sp_d, sp_l)
    inp("b_gate", (3, 1), F32)
    # fusion
    inp("w_fuse", (TOTAL, TOTAL), BF16)
    inp("bias_fuse", (P, 24), F32)
    # one-hot selector: sel3[:, m*128:(m+1)*128] broadcasts row m of a [3, N]
    # rhs to all 128 output partitions
    inp("sel3", (3, 3 * P), BF16)

    p["out"] = nc.dram_tensor("out", [TOTAL, NT], F32, kind="ExternalOutput").ap()
    return p


def _emit(nc, tc, p, ctx):
    mm = nc.tensor.matmul

    dram = ctx.enter_context(tc.tile_pool(name="dram", bufs=1, space="DRAM"))
    psum = ctx.enter_context(tc.tile_pool(name="psum", bufs=1, space="PSUM"))
    const = ctx.enter_context(tc.tile_pool(name="const", bufs=1))
    sb = ctx.enter_context(tc.tile_pool(name="sb", bufs=1))

    # ---- constants ----
    ones_col_bf = const.tile([P, 1], BF16, name="ones_col_bf")
    nc.vector.memset(ones_col_bf, 1.0)
    inv16_col_bf = const.tile([P, 1], BF16, name="inv16_col_bf")
    nc.vector.memset(inv16_col_bf, 1.0 / ASCALE)
    ones_row_bf = const.tile([1, P], BF16, name="ones_row_bf")
    nc.vector.memset(ones_row_bf, 1.0)

    def load_const(name, ap):
        t = const.tile(list(ap.shape), ap.dtype, name=name)
        nc.scalar.dma_start(t, ap)
        return t

    bias_proj = load_const("bias_proj", p["bias_proj"])
    bias_v = load_const("bias_v", p["bias_v"])
    bias_o = load_const("bias_o", p["bias_o"])
    desc = load_const("desc", p["desc"])
    b_se1 = load_const("b_se1", p["b_se1"])
    b_se2 = load_const("b_se2", p["b_se2"])
    b_gate = load_const("b_gate", p["b_gate"])
    bias_fuse = load_const("bias_fuse", p["bias_fuse"])
    w_gate_x = load_const("w_gate_x", p["w_gate_x"])
    sel3 = load_const("sel3", p["sel3"])

    def dma_slab(dst, src, kp=P, eng=None, split=False):
        """DMA [(kt p), n] DRAM weight into [p, kt*n] SBUF slab.

        split=True issues one DMA per k-tile round-robin across queues so a
        single queue's bandwidth doesn't gate the load."""
        kt = src.shape[0] // kp
        d = dst.rearrange("p (kt n) -> p kt n", kt=kt)
        s = src.rearrange("(kt p) n -> p kt n", p=kp)
        if split:
            rr = [nc.sync, nc.scalar]
            for j in range(kt):
                rr[j % 2].dma_start(d[:, j:j + 1, :], s[:, j:j + 1, :])
        else:
            (eng or nc.sync).dma_start(d, s)

    # small resident weights
    w_gate_s = const.tile([P, 24 * 3], BF16, name="w_gate_s")
    nc.gpsimd.dma_start(w_gate_s.rearrange("p (kt n) -> p kt n", kt=24),
                        p["w_gate"].rearrange("(kt p) n -> p kt n", p=P))
    w_se1_s = const.tile([P, 3 * 8 * SEH], BF16, name="w_se1_s")
    nc.gpsimd.dma_start(
        w_se1_s.rearrange("p (m kt n) -> p m kt n", m=3, kt=8),
        p["w_se1"].rearrange("m (kt p) n -> p m kt n", p=P))

    # warm up the collectives path (first AllReduce pays ~10us extra)
    cw_in = dram.tile([1, 16], F32, name="ccwarm_in", tag="ccwarm_in")
    cw_out = dram.tile([1, 16], F32, name="ccwarm_out", tag="ccwarm_out")
    warm_z = const.tile([1, 16], F32, name="warm_z")
    nc.vector.memset(warm_z, 0.0)
    nc.gpsimd.dma_start(cw_in, warm_z)
    nc.gpsimd.collective_compute(
        "AllReduce", ALU.add, replica_groups=RG_PAIR,
        ins=[cw_in.opt()], outs=[cw_out.opt()])

    # ---------------- Phase A: projections + L2 normalize ----------------
    # rdl[m][ct] holds (normalized features + o-bias) = residual tiles.
    # f8[m][pair] holds fp8 copies (x ASCALE) of the normalized features.
    rdl = [[None] * 8 for _ in range(3)]
    f8 = [[None] * 4 for _ in range(3)]
    mod_meta = [("rgb", 4, p["x_rgb"], p["w_rgb"]),
                ("depth", 2, p["x_depth"], p["w_depth"]),
                ("lidar", 1, p["x_lidar"], p["w_lidar"])]

    xin = {}
    for mi in (1, 0, 2):
        mname, nkt, x_ap, _ = mod_meta[mi]
        kp = P if mname != "lidar" else 64
        for kt in range(nkt):
            t = sb.tile([kp, NT], BF16, name=f"x_{mname}{kt}", tag="v", bufs=20)
            nc.gpsimd.dma_start(t, x_ap[kt * P:kt * P + kp, :])
            xin[(mi, kt)] = t

    proj_t = {}
    sinv_t = {}

    def proj_mm(mi):
        mname, nkt, _, w_ap = mod_meta[mi]
        kp = P if mname != "lidar" else 64
        wtag, wbufs = ("wlid", 1) if mname == "lidar" else ("wproj", 2)
        ws = sb.tile([kp, nkt * BC], BF16, name=f"w_{mname}_s", tag=wtag, bufs=wbufs)
        dma_slab(ws, w_ap, kp=kp, split=True)
        proj = []
        ss_ps = psum.tile([1, NT], F32, name=f"ss_{mname}", tag="pat", bufs=2)
        for ct in range(8):
            ps = psum.tile([P, NT], F32, name=f"pp_{mname}{ct}", tag="pmm", bufs=3)
            for kt in range(nkt):
                mm(ps, ws[:, kt * BC + ct * P: kt * BC + (ct + 1) * P],
                   xin[(mi, kt)], start=(kt == 0), stop=(kt == nkt - 1))
            pt = sb.tile([P, NT], BF16, name=f"proj_{mname}{ct}", tag="v", bufs=20)
            nc.scalar.activation(pt, ps, AF.Identity,
                                 bias=bias_proj[:, mi * 8 + ct: mi * 8 + ct + 1])
            proj.append(pt)
            sq = sb.tile([P, NT], BF16, name=f"sq_{mname}{ct}", tag="sq", bufs=2)
            nc.scalar.activation(sq, pt, AF.Square)
            mm(ss_ps, ones_col_bf, sq, start=(ct == 0), stop=(ct == 7))
        snorm = sb.tile([1, NT], F32, name=f"snorm_{mname}", tag="row", bufs=4)
        nc.scalar.activation(snorm, ss_ps, AF.Sqrt)
        sinv = sb.tile([1, NT], BF16, name=f"sinv_{mname}", tag="row", bufs=4)
        with nc.allow_low_precision(reason="1/norm broadcast as bf16 matmul rhs"):
            nc.vector.reciprocal(sinv, snorm)
        proj_t[mi] = proj
        sinv_t[mi] = sinv

    def norm_mul(mi):
        mname = mod_meta[mi][0]
        bc_ps = psum.tile([P, NT], F32, name=f"bc_{mname}", tag="pmm", bufs=3)
        mm(bc_ps, ones_row_bf, sinv_t[mi])
        proj = proj_t[mi]
        for ct in range(8):
            rt = sb.tile([P, NT], BF16, name=f"f_{mname}{ct}", tag="rdl", bufs=24)
            nc.vector.tensor_mul(rt, proj[ct], bc_ps)
            if ct % 2 == 0:
                f8[mi][ct // 2] = sb.tile([P, 2 * NT], FP8,
                                          name=f"f8_{mname}{ct // 2}",
                                          tag="f8", bufs=12)
            # fp8 copy (x ASCALE) for the QKV GEMM inputs
            nc.vector.tensor_scalar_mul(
                f8[mi][ct // 2][:, (ct % 2) * NT:(ct % 2 + 1) * NT],
                rt, ASCALE)
            # fold the (pair-summed) o-bias into the residual tile
            nc.vector.tensor_scalar_add(
                rt, rt, bias_o[:, mi * 8 + ct: mi * 8 + ct + 1])
            rdl[mi][ct] = rt

    # interleave: projections keep the PE busy while norm chains resolve
    proj_mm(1)          # depth
    proj_mm(0)          # rgb
    norm_mul(1)
    proj_mm(2)          # lidar
    norm_mul(0)
    norm_mul(2)

    # ---------------- attention ----------------
    def qkv_phase(i):
        qm, km = BLOCKS[i]
        f8q, f8kv = f8[qm], f8[km]

        def w8_slab(ap, label, eng):
            t = sb.tile([P, 8192], FP8, name=f"{label}{i}", tag="wbig", bufs=4)
            eng.dma_start(t, ap)
            return t.rearrange("p (ktp g n) -> p ktp g n", ktp=4, g=2)

        wv = w8_slab(p["w8_v"][i], "wv", nc.sync)
        wk = w8_slab(p["w8_k"][i], "wk", nc.scalar)
        wq = w8_slab(p["w8_q"][i], "wq", nc.sync)

        v_cm = []
        for ct in range(8):
            ps = psum.tile([P, NT], F32, name=f"ps_v{i}_{ct}", tag="pmm", bufs=3)
            for ktp in range(4):
                mm(ps, wv[:, ktp, :, ct * P:(ct + 1) * P],
                   f8kv[ktp].rearrange("p (g n) -> p g n", g=2),
                   start=(ktp == 0), stop=(ktp == 3), perf_mode=DR)
            vt = sb.tile([P, NT], BF16, name=f"v{i}_{ct}", tag="v", bufs=20)
            nc.scalar.activation(vt, ps, AF.Identity,
                                 scale=desc[:, 12 + i:13 + i],
                                 bias=bias_v[:, i * 8 + ct: i * 8 + ct + 1])
            v_cm.append(vt)

        def tok_major(ws, dcol, bseg, label, f8m):
            bbc = sb.tile([P, BC], BF16, name=f"b_{label}{i}", tag="qkbc", bufs=2)
            nc.scalar.dma_start(bbc, p["qk_bias"][bseg])
            tiles = []
            for mt in range(4):
                t = sb.tile([P, BC], BF16, name=f"{label}{i}m{mt}", tag="qk", bufs=9)
                for nt2 in range(2):
                    ps = psum.tile([P, 512], F32, name=f"ps_{label}{i}_{mt}{nt2}",
                                   tag="pmm", bufs=3)
                    for ktp in range(4):
                        mm(ps,
                           f8m[ktp].rearrange("p (g n) -> p g n", g=2)[
                               :, :, mt * P:(mt + 1) * P],
                           ws[:, ktp, :, nt2 * 512:(nt2 + 1) * 512],
                           start=(ktp == 0), stop=(ktp == 3), perf_mode=DR)
                    nc.vector.scalar_tensor_tensor(
                        t[:, nt2 * 512:(nt2 + 1) * 512], ps,
                        desc[:, dcol:dcol + 1],
                        bbc[:, nt2 * 512:(nt2 + 1) * 512],
                        op0=ALU.mult, op1=ALU.add)
                tiles.append(t)
            return tiles

        k_tm = tok_major(wk, 6 + i, 2 * i + 1, "k", f8kv)
        q_tm = tok_major(wq, i, 2 * i, "q", f8q)

        # scoresT[k, h] partials: contract over the 512 local positions
        stage = sb.tile([P, HEADS * P], BF16, name=f"sstage{i}", tag="sc", bufs=3)
        for h in range(HEADS):
            ps = psum.tile([P, P], F32, name=f"ps_sc{i}_{h}", tag="psc", bufs=2)
            for mt in range(4):
                mm(ps, k_tm[mt][:, h * HD:(h + 1) * HD],
                   q_tm[mt][:, h * HD:(h + 1) * HD],
                   start=(mt == 0), stop=(mt == 3))
            nc.vector.tensor_copy(stage[:, h * P:(h + 1) * P], ps)

        cc_in = dram.tile([P, HEADS * P], BF16, name=f"ccin{i}", tag=f"ccin{i}")
        cc_out = dram.tile([P, HEADS * P], BF16, name=f"ccout{i}", tag=f"ccout{i}")
        nc.gpsimd.dma_start(cc_in, stage)
        nc.gpsimd.collective_compute(
            "AllReduce", ALU.add, replica_groups=RG_PAIR,
            ins=[cc_in.opt()], outs=[cc_out.opt()])
        gather = sb.tile([P, HEADS * P], BF16, name=f"sgather{i}", tag="sc", bufs=3)
        nc.gpsimd.dma_start(gather, cc_out)
        return v_cm, gather

    def attn_phase(i, v_cm, gather):
        """softmax (no max-subtract; |logits| < 0.02) + attn @ V -> fp8 pairs."""
        ao8 = []
        for h in range(HEADS):
            ex = sb.tile([P, P], BF16, name=f"ex{i}_{h}", tag="ex", bufs=4)
            nc.scalar.activation(ex, gather[:, h * P:(h + 1) * P], AF.Exp,
                                 scale=SCALE)
            ps = psum.tile([P, NT], F32, name=f"ps_at{i}_{h}", tag="pat", bufs=2)
            mm(ps, ex, v_cm[h])
            cs = psum.tile([P, 1], F32, name=f"cs{i}_{h}", tag="psc", bufs=2)
            mm(cs, ex, inv16_col_bf)          # sum(exp)/ASCALE
            rec = sb.tile([P, 1], F32, name=f"rec{i}_{h}", tag="rec", bufs=4)
            nc.vector.reciprocal(rec, cs)     # ASCALE / sum(exp)
            if h % 2 == 0:
                t8 = sb.tile([P, 2 * NT], FP8, name=f"ao8_{i}_{h // 2}",
                             tag="ao8", bufs=10)
                ao8.append(t8)
            nc.vector.tensor_scalar_mul(
                ao8[h // 2][:, (h % 2) * NT:(h % 2 + 1) * NT], ps, rec)
        return ao8

    def o_pair(m, ao_even, ao_odd):
        st = sb.tile([P, 8], F32, name=f"se_st{m}", tag=f"se_a{m}", bufs=1)
        i0, i1 = 2 * m, 2 * m + 1
        wo0 = sb.tile([P, 8192], FP8, name=f"wo{i0}", tag="wbig", bufs=4)
        nc.sync.dma_start(wo0, p["w8_o"][i0])
        wo0r = wo0.rearrange("p (ktp g n) -> p ktp g n", ktp=4, g=2)
        wo1 = sb.tile([P, 8192], FP8, name=f"wo{i1}", tag="wbig", bufs=4)
        nc.scalar.dma_start(wo1, p["w8_o"][i1])
        wo1r = wo1.rearrange("p (ktp g n) -> p ktp g n", ktp=4, g=2)
        cross = []
        for ct in range(8):
            ps = psum.tile([P, NT], F32, name=f"ps_o{m}_{ct}", tag="pmm", bufs=3)
            for ktp in range(4):
                mm(ps, wo0r[:, ktp, :, ct * P:(ct + 1) * P],
                   ao_even[ktp].rearrange("p (g n) -> p g n", g=2),
                   start=(ktp == 0), stop=False, perf_mode=DR)
            for ktp in range(4):
                mm(ps, wo1r[:, ktp, :, ct * P:(ct + 1) * P],
                   ao_odd[ktp].rearrange("p (g n) -> p g n", g=2),
                   start=False, stop=(ktp == 3), perf_mode=DR)
            crt = sb.tile([P, NT], BF16, name=f"cross{m}_{ct}", tag="cross", bufs=24)
            # desc * psum + (residual + pair-summed o-bias); accum_out gives
            # the SE global-average-pool sum for free
            nc.vector.scalar_tensor_tensor(
                crt, ps, desc[:, 18 + m:19 + m], rdl[m][ct],
                op0=ALU.mult, op1=ALU.add, accum_out=st[:, ct:ct + 1])
            cross.append(crt)
        # launch the SE pool AllReduce immediately
        se_in = dram.tile([P, 8], F32, name=f"se_ccin{m}", tag=f"se_ccin{m}")
        se_out = dram.tile([P, 8], F32, name=f"se_ccout{m}", tag=f"se_ccout{m}")
        nc.gpsimd.dma_start(se_in, st)
        nc.gpsimd.collective_compute(
            "AllReduce", ALU.add, replica_groups=RG_PAIR,
            ins=[se_in.opt()], outs=[se_out.opt()])
        return cross, se_out

    gse = [None] * 3
    wg_eff = sb.tile([P, 24 * 3], BF16, name="wg_eff", tag="wge", bufs=1)

    def se_finish(m, se_out):
        """SE MLP for modality m (AllReduce was launched by o_pair); folds
        the SE gate into the gate-conv weights (per-core batch is fixed, so
        the gate is a per-channel scalar)."""
        pf = sb.tile([P, 8], F32, name=f"se_pf{m}", tag=f"se_b{m}", bufs=1)
        nc.gpsimd.dma_start(pf, se_out)
        pb = sb.tile([P, 8], BF16, name=f"se_pb{m}", tag=f"se_c{m}", bufs=1)
        nc.vector.tensor_copy(pb, pf)
        h_ps = psum.tile([SEH, 1], F32, name=f"h_ps{m}", tag="psc", bufs=2)
        for kt in range(8):
            mm(h_ps, w_se1_s[:, (m * 8 + kt) * SEH: (m * 8 + kt + 1) * SEH],
               pb[:, kt:kt + 1], start=(kt == 0), stop=(kt == 7))
        h_sb = sb.tile([SEH, 1], BF16, name=f"h_sb{m}", tag="rec", bufs=4)
        nc.scalar.activation(h_sb, h_ps, AF.Relu, bias=b_se1[:, m:m + 1])
        sew = sb.tile([SEH, BC], BF16, name=f"sew{m}", tag="sew", bufs=2)
        nc.gpsimd.dma_start(sew, p["w_se2"][m])
        gm = sb.tile([P, 8], F32, name=f"gse{m}", tag="gse", bufs=3)
        for ct in range(8):
            g_ps = psum.tile([P, 1], F32, name=f"g_ps{m}_{ct}", tag="psc", bufs=2)
            mm(g_ps, sew[:, ct * P:(ct + 1) * P], h_sb)
            nc.scalar.activation(gm[:, ct:ct + 1], g_ps, AF.Sigmoid,
                                 bias=b_se2[:, m * 8 + ct: m * 8 + ct + 1])
        gse[m] = gm
        for ct in range(8):
            kt = m * 8 + ct
            nc.vector.tensor_scalar_mul(wg_eff[:, kt * 3:(kt + 1) * 3],
                                        w_gate_s[:, kt * 3:(kt + 1) * 3],
                                        gm[:, ct:ct + 1])

    def sparsity_metric(mq):
        sp_ps = psum.tile([1, NT], F32, name=f"sp_ps{mq}", tag="pat", bufs=2)
        for ct in range(8):
            msk = sb.tile([P, NT], BF16, name=f"msk{mq}_{ct}", tag="sq", bufs=2)
            # rdl == bias_o  <=>  normalized feature == 0
            nc.vector.tensor_scalar(
                msk, rdl[mq][ct], bias_o[:, mq * 8 + ct: mq * 8 + ct + 1],
                None, op0=ALU.is_equal)
            mm(sp_ps, ones_col_bf, msk, start=(ct == 0), stop=(ct == 7))
        sp_row = sb.tile([1, NT], BF16, name=f"sp_row{mq}", tag="row", bufs=4)
        nc.scalar.activation(sp_row, sp_ps, AF.Copy, scale=1.0 / 1024.0)
        return sp_row

    # pipelined blocks: AllReduce(i) overlaps the next QKV phase; SE chains
    # are emitted after independent PE work so their AllReduce never stalls
    # the in-order PE queue
    v0, g0 = qkv_phase(0)
    v1, g1 = qkv_phase(1)
    ao0 = attn_phase(0, v0, g0)
    v2, g2 = qkv_phase(2)
    ao1 = attn_phase(1, v1, g1)
    cross_rgb, seo0 = o_pair(0, ao0, ao1)
    v3, g3 = qkv_phase(3)
    ao2 = attn_phase(2, v2, g2)
    se_finish(0, seo0)
    v4, g4 = qkv_phase(4)
    ao3 = attn_phase(3, v3, g3)
    cross_dep, seo1 = o_pair(1, ao2, ao3)
    v5, g5 = qkv_phase(5)
    ao4 = attn_phase(4, v4, g4)

    # quality metrics (from the rdl tiles = normalized features + o-bias;
    # the o-bias shift is compensated exactly) - fill the SE wait
    rsum_ps = psum.tile([1, NT], F32, name="rsum_ps", tag="pat", bufs=2)
    for ct in range(8):
        mm(rsum_ps, ones_col_bf, rdl[0][ct], start=(ct == 0), stop=(ct == 7))
    # var = 1/1023 - (rsum' - C)^2/(1024*1023)  with C = sum(bias_o_rgb)
    xr = []
    var_row = sb.tile([1, NT], BF16, name="var_row", tag="row", bufs=4)
    nc.scalar.activation(var_row, rsum_ps, AF.Square,
                         scale=float(1.0 / np.sqrt(1024.0 * 1023.0)),
                         bias=desc[0:1, 21:22])
    nc.vector.tensor_scalar(var_row, var_row, -1.0, 1.0 / 1023.0,
                            op0=ALU.mult, op1=ALU.add)
    xr.append(var_row)
    xr.append(sparsity_metric(1))

    se_finish(1, seo1)
    crosses_rd = [cross_rgb, cross_dep]

    # rgb/depth contributions to the gate conv - filler while AR(5) flies
    gt_ps = psum.tile([3, NT], F32, name="gt_ps", tag="prow", bufs=1)
    for m in range(2):
        for ct in range(8):
            kt = m * 8 + ct
            mm(gt_ps, wg_eff[:, kt * 3:(kt + 1) * 3], crosses_rd[m][ct],
               start=(kt == 0), stop=False)

    ao5 = attn_phase(5, v5, g5)
    cross_lid, seo2 = o_pair(2, ao4, ao5)
    xr.append(sparsity_metric(2))      # fills the SE(2) AllReduce wait
    se_finish(2, seo2)
    crosses = [cross_rgb, cross_dep, cross_lid]

    # lidar part + extras of the gate conv; rgb/depth parts were emitted early
    for ct in range(8):
        mm(gt_ps, wg_eff[:, (16 + ct) * 3:(17 + ct) * 3], crosses[2][ct],
           start=False, stop=False)
    for m in range(3):
        mm(gt_ps, w_gate_x[0:1, 3 * m:3 * m + 3], xr[m],
           start=False, stop=(m == 2))
    gates = sb.tile([3, NT], BF16, name="gates", tag="row", bufs=4)
    nc.scalar.activation(gates, gt_ps, AF.Sigmoid, bias=b_gate[:, 0:1])

    # ---------------- Phase E: fused features + fusion conv ----------------
    # fused = cross * se_gate * modality_gate, in place in one pass
    for m in range(3):
        bc_ps = psum.tile([P, NT], F32, name=f"gbc{m}", tag="pmm", bufs=3)
        mm(bc_ps, sel3[:, m * P:(m + 1) * P], gates)
        for ct in range(8):
            nc.vector.scalar_tensor_tensor(
                crosses[m][ct], crosses[m][ct], gse[m][:, ct:ct + 1], bc_ps,
                op0=ALU.mult, op1=ALU.mult)
    fused = [crosses[m][ct] for m in range(3) for ct in range(8)]

    for ct in range(24):
        wf = sb.tile([P, 24 * P], BF16, name=f"wf{ct}", tag="wfuse", bufs=2)
        dma_slab(wf, p["w_fuse"][:, ct * P:(ct + 1) * P])
        ps = psum.tile([P, NT], F32, name=f"ps_f{ct}", tag="pmm", bufs=3)
        for kt in range(24):
            mm(ps, wf[:, kt * P:(kt + 1) * P], fused[kt],
               start=(kt == 0), stop=(kt == 23))
        ot = sb.tile([P, NT], F32, name=f"ot{ct}", tag="outb", bufs=2)
        nc.scalar.activation(ot, ps, AF.Identity, bias=bias_fuse[:, ct: ct + 1])
        nc.gpsimd.dma_start(p["out"][ct * P:(ct + 1) * P, :], ot)


def _build():
    nc = bacc.Bacc("TRN2", target_bir_lowering=False, debug=False,
                   num_devices=NCORES)
    params = _declare_params(nc)
    with tile.TileContext(nc) as tc, ExitStack() as ctx:
        _emit(nc, tc, params, ctx)
    nc.compile()
    return nc


def _fp8_slab(wT, s):
    """[1024, 1024] (already [Cin, Cout]) -> [128, 8192] fp8 DoubleRow image."""
    q8 = (wT * s).astype(e4m3)
    return np.ascontiguousarray(
        q8.reshape(4, 2, P, BC).transpose(2, 0, 1, 3).reshape(P, 8192))


def _pow2_scale(w):
    """largest power of 2 with max|w|*s <= 224"""
    m = float(np.abs(w).max())
    return 2.0 ** np.floor(np.log2(224.0 / m))


def _prep_static(inputs):
    """Host-side weight prep shared by all cores."""
    f32 = np.float32

    def colpack(b):  # [1024] -> [128, 8] (col j = channel tile j)
        return np.ascontiguousarray(b.reshape(8, P).T.astype(f32))

    s = {}
    s["w_rgb"] = inputs["proj_rgb_w"].T.astype(bf16)
    s["w_depth"] = inputs["proj_depth_w"].T.astype(bf16)
    s["w_lidar"] = inputs["proj_lidar_w"].T.astype(bf16)

    desc = np.zeros((P, 24), f32)
    for nm, base in (("q", 0), ("k", 6), ("v", 12)):
        w = inputs[f"attn_{nm}_w"]
        slabs = np.empty((6, P, 8192), e4m3)
        for i in range(6):
            wT = np.ascontiguousarray(w[i].T)
            sc = _pow2_scale(wT)
            slabs[i] = _fp8_slab(wT, sc)
            desc[:, base + i] = 1.0 / (sc * ASCALE)
        s[f"w8_{nm}"] = slabs
    wo = inputs["attn_o_w"]
    slabs = np.empty((6, P, 8192), e4m3)
    for m in range(3):
        sc = min(_pow2_scale(wo[2 * m].T), _pow2_scale(wo[2 * m + 1].T))
        slabs[2 * m] = _fp8_slab(np.ascontiguousarray(wo[2 * m].T), sc)
        slabs[2 * m + 1] = _fp8_slab(np.ascontiguousarray(wo[2 * m + 1].T), sc)
        desc[:, 18 + m] = 1.0 / (sc * ASCALE)
    s["w8_o"] = slabs
    # var-metric bias: -sum(bias_o_rgb)/sqrt(1024*1023), from the
    # bf16-rounded bias actually folded into the rdl tiles
    bo_rgb = (inputs["attn_o_b"][0] + inputs["attn_o_b"][1]) \
        .astype(bf16).astype(np.float64)
    desc[0, 21] = float(-bo_rgb.sum() / np.sqrt(1024.0 * 1023.0))
    s["desc"] = desc

    qk = np.empty((12, BC), f32)
    for i in range(6):
        qk[2 * i] = inputs["attn_q_b"][i]
        qk[2 * i + 1] = inputs["attn_k_b"][i]
    s["qk_bias"] = np.ascontiguousarray(
        np.broadcast_to(qk[:, None, :], (12, P, BC))).astype(bf16)
    s["bias_proj"] = np.concatenate(
        [colpack(inputs["proj_rgb_b"]), colpack(inputs["proj_depth_b"]),
         colpack(inputs["proj_lidar_b"])], axis=1)
    s["bias_v"] = np.concatenate(
        [colpack(inputs["attn_v_b"][i]) for i in range(6)], axis=1)
    # round the pair-summed o-bias through bf16 so that the on-device
    # is_equal sparsity test (rdl == bias_o) is exact for true zeros
    s["bias_o"] = np.concatenate(
        [colpack((inputs["attn_o_b"][2 * m] + inputs["attn_o_b"][2 * m + 1])
                 .astype(bf16).astype(np.float32))
         for m in range(3)], axis=1)
    s["w_se1"] = np.ascontiguousarray(
        (inputs["se_w1"] / 1024.0).transpose(0, 2, 1)).astype(bf16)
    s["b_se1"] = np.ascontiguousarray(inputs["se_b1"].T.astype(f32))
    s["w_se2"] = np.ascontiguousarray(inputs["se_w2"].transpose(0, 2, 1)).astype(bf16)
    s["b_se2"] = np.concatenate(
        [colpack(inputs["se_b2"][m]) for m in range(3)], axis=1)
    s["w_gate"] = np.ascontiguousarray(inputs["gate_w"][:, :TOTAL].T).astype(bf16)
    s["w_gate_x"] = np.ascontiguousarray(
        inputs["gate_w"][:, TOTAL:].T.astype(f32)).reshape(1, 9).astype(bf16)
    s["b_gate"] = inputs["gate_b"].reshape(3, 1).astype(f32)
    s["w_fuse"] = np.ascontiguousarray(inputs["fusion_w"].T).astype(bf16)
    s["bias_fuse"] = np.concatenate(
        [colpack(inputs["fusion_b"][ct * BC:(ct + 1) * BC]) for ct in range(3)],
        axis=1)
    sel = np.zeros((3, 3 * P), bf16)
    for m in range(3):
        sel[m, m * P:(m + 1) * P] = 1.0
    s["sel3"] = sel
    return s


def kernel(**inputs):
    global _compiled, LAST_RESULTS
    if _compiled is None:
        _compiled = _build()
    nc = _compiled

    static = _prep_static(inputs)

    def shard(x, c):  # core c: batch c//2, position half c%2, channel-major
        b, hf = c // 2, c % 2
        C = x.shape[1]
        return np.ascontiguousarray(
            x.reshape(B, C, HW)[b, :, hf * NT:(hf + 1) * NT]).astype(bf16)

    in_maps = []
    for c in range(NCORES):
        m = dict(static)
        m["x_rgb"] = shard(inputs["rgb_features"], c)
        m["x_depth"] = shard(inputs["depth_features"], c)
        m["x_lidar"] = shard(inputs["lidar_features"], c)
        in_maps.append(m)

    res = run_bass_kernel_spmd(nc, in_maps, core_ids=list(range(NCORES)),
                               trace=TRACE)
    LAST_RESULTS = res

    full = np.empty((B, TOTAL, HW), np.float32)
    for c in range(NCORES):
        b, hf = c // 2, c % 2
        full[b, :, hf * NT:(hf + 1) * NT] = res.results[c]["out"]
    return full.reshape(B, TOTAL, 32, 32)


# revision 23
# speedup vs baseline: 1.1100x; 1.1100x over previous
"""Trainium2 distributed Bass kernel for AdaptiveGatedAttentionFusion.

Sharding: each of the 8 cores owns (batch b = core//2, half hf = core%2 of the
1024 spatial positions) -> 512 tokens per core. All weights replicated.
Activations channel-major [C, pos] in SBUF.

v2: the 24 heavy QKV/O 1024x1024 GEMMs run in fp8-e4m3 DoubleRow mode
(256-deep contraction per matmul, ~1.7x TensorE streaming) with fp32 PSUM
accumulation and data-driven descale factors.  Normalized features are
provably bounded (per-token L2 norm == 1), so fixed fp8 scales cannot
overflow.  Projections / fusion / scores / attn@V stay bf16 (fp8 there blows
the error budget - measured by simulation).

Cross-core communication (pairwise, cores (2b, 2b+1) share batch b):
  - one AllReduce per attention block for the head_dim x head_dim score
    matrices (spatial contraction split across the pair), overlapped with
    the next block's QKV projections.
  - one tiny AllReduce per modality for the SE global-average-pool sums.
"""

from contextlib import ExitStack

import numpy as np
import ml_dtypes

import concourse.bacc as bacc
import concourse.bass as bass
import concourse.mybir as mybir
import concourse.tile as tile
from concourse.bass_utils import run_bass_kernel_spmd

F32 = mybir.dt.float32
BF16 = mybir.dt.bfloat16
FP8 = mybir.dt.float8e4
AF = mybir.ActivationFunctionType
ALU = mybir.AluOpType
DR = mybir.MatmulPerfMode.DoubleRow

NCORES = 8
B = 4
HW = 1024
NT = 512                    # tokens per core (half the positions of one batch)
BC = 1024
HEADS = 8
HD = 128
SEH = 64                    # BC / reduction(16)
TOTAL = 3 * BC
SCALE = float(HD) ** -0.5
P = 128
ASCALE = 16.0               # fp8 activation scale (|act| <= ~4 everywhere)

RGB_C, DEPTH_C, LIDAR_C = 512, 256, 64
# attention block wiring: (q_modality, kv_modality); 0=rgb 1=depth 2=lidar
BLOCKS = [(0, 1), (0, 2), (1, 0), (1, 2), (2, 0), (2, 1)]
# pairwise replica groups: cores (2b, 2b+1) hold the two halves of batch b
RG_PAIR = [[0, 1], [2, 3], [4, 5], [6, 7]]

bf16 = ml_dtypes.bfloat16
e4m3 = ml_dtypes.float8_e4m3   # TRN FP8_EXP4-compatible (bias 7, max 240)

_compiled = None
LAST_RESULTS = None       # BassKernelResults of last run (for test harness)
TRACE = False


def _declare_params(nc):
    p = {}

    def inp(name, shape, dt):
        p[name] = nc.dram_tensor(name, list(shape), dt, kind="ExternalInput").ap()

    # per-core activation shard, channel-major [C, NT]
    inp("x_rgb", (RGB_C, NT), BF16)
    inp("x_depth", (DEPTH_C, NT), BF16)
    inp("x_lidar", (LIDAR_C, NT), BF16)
    # projection weight slabs, prepacked SBUF image [p, kt, n] (contiguous)
    inp("w_rgb", (P, 4 * BC), BF16)
    inp("w_depth", (P, 2 * BC), BF16)
    inp("w_lidar", (64, BC), BF16)
    # attention weights: fp8 DoubleRow slabs, SBUF image layout
    # slab[p, ktp*2048 + g*1024 + n] = (w.T)[ktp*256 + g*128 + p, n] * s_w
    inp("w8_q", (6, P, 8192), FP8)
    inp("w8_k", (6, P, 8192), FP8)
    inp("w8_v", (6, P, 8192), FP8)
    inp("w8_o", (6, P, 8192), FP8)
    # q/k biases pre-broadcast to all 128 partitions (row 2i = q_b[i], 2i+1 = k_b[i])
    inp("qk_bias", (12, P, BC), BF16)
    # per-partition bias columns, f32, col j = channel tile j
    inp("bias_proj", (P, 24), F32)     # rgb 0:8, depth 8:16, lidar 16:24
    inp("bias_v", (P, 48), F32)        # block i cols 8i:8i+8
    inp("bias_o", (P, 24), F32)        # modality m cols 8m:8m+8 (pair-summed)
    # fp8 descale factors (per-partition broadcast columns):
    # cols 0-5 q[i], 6-11 k[i], 12-17 v[i], 18-20 o[m], col 21 row0 = var bias
    inp("desc", (P, 24), F32)
    # SE
    inp("w_se1", (3, BC, SEH), BF16)   # (se_w1/1024).T per modality
    inp("b_se1", (SEH, 3), F32)
    inp("w_se2", (3, SEH, BC), BF16)   # se_w2.T
    inp("b_se2", (P, 24), F32)         # modality m cols 8m:8m+8
    # gate
    inp("w_gate", (TOTAL, 3), BF16)    # gate_w[:, :3072].T
    inp("w_gate_x", (1, 9), BF16)      # seg m = gate_w[:, 3072+m] (var, sp_d, sp_l)
    inp("b_gate", (3, 1), F32)
    # fusion weight slabs, prepacked [p, ct, kt, n] (contiguous per ct)
    inp("w_fuse", (P, 24 * TOTAL), BF16)
    inp("bias_fuse", (P, 24), F32)
    # one-hot selector: sel3[:, m*128:(m+1)*128] broadcasts row m of a [3, N]
    # rhs to all 128 output partitions
    inp("sel3", (3, 3 * P), BF16)

    p["out"] = nc.dram_tensor("out", [TOTAL, NT], F32, kind="ExternalOutput").ap()
    return p


def _emit(nc, tc, p, ctx):
    mm = nc.tensor.matmul

    dram = ctx.enter_context(tc.tile_pool(name="dram", bufs=1, space="DRAM"))
    psum = ctx.enter_context(tc.tile_pool(name="psum", bufs=1, space="PSUM"))
    const = ctx.enter_context(tc.tile_pool(name="const", bufs=1))
    sb = ctx.enter_context(tc.tile_pool(name="sb", bufs=1))

    # ---- constants ----
    ones_col_bf = const.tile([P, 1], BF16, name="ones_col_bf")
    nc.vector.memset(ones_col_bf, 1.0)
    inv16_col_bf = const.tile([P, 1], BF16, name="inv16_col_bf")
    nc.vector.memset(inv16_col_bf, 1.0 / ASCALE)
    ones_row_bf = const.tile([1, P], BF16, name="ones_row_bf")
    nc.vector.memset(ones_row_bf, 1.0)

    def load_const(name, ap):
        t = const.tile(list(ap.shape), ap.dtype, name=name)
        nc.scalar.dma_start(t, ap)
        return t

    bias_proj = load_const("bias_proj", p["bias_proj"])
    bias_v = load_const("bias_v", p["bias_v"])
    bias_o = load_const("bias_o", p["bias_o"])
    desc = load_const("desc", p["desc"])
    b_se1 = load_const("b_se1", p["b_se1"])
    b_se2 = load_const("b_se2", p["b_se2"])
    b_gate = load_const("b_gate", p["b_gate"])
    bias_fuse = load_const("bias_fuse", p["bias_fuse"])
    w_gate_x = load_const("w_gate_x", p["w_gate_x"])
    sel3 = load_const("sel3", p["sel3"])

    def dma_split(dst, src):
        """prepacked slab load, halves on separate queues"""
        h = src.shape[-1] // 2
        nc.sync.dma_start(dst[:, :h], src[:, :h])
        nc.scalar.dma_start(dst[:, h:], src[:, h:])

    # warm up the collectives path (first AllReduce pays ~10us extra)
    cw_in = dram.tile([1, 16], F32, name="ccwarm_in", tag="ccwarm_in")
    cw_out = dram.tile([1, 16], F32, name="ccwarm_out", tag="ccwarm_out")
    warm_z = const.tile([1, 16], F32, name="warm_z")
    nc.vector.memset(warm_z, 0.0)
    nc.gpsimd.dma_start(cw_in, warm_z)
    nc.gpsimd.collective_compute(
        "AllReduce", ALU.add, replica_groups=RG_PAIR,
        ins=[cw_in.opt()], outs=[cw_out.opt()])

    # ---------------- Phase A: projections + L2 normalize ----------------
    # rdl[m][ct] holds (normalized features + o-bias) = residual tiles.
    # f8[m][pair] holds fp8 copies (x ASCALE) of the normalized features.
    rdl = [[None] * 8 for _ in range(3)]
    f8 = [[None] * 4 for _ in range(3)]
    mod_meta = [("rgb", 4, p["x_rgb"], p["w_rgb"]),
                ("depth", 2, p["x_depth"], p["w_depth"]),
                ("lidar", 1, p["x_lidar"], p["w_lidar"])]

    xin = {}
    for mi in (1, 0, 2):
        mname, nkt, x_ap, _ = mod_meta[mi]
        kp = P if mname != "lidar" else 64
        for kt in range(nkt):
            t = sb.tile([kp, NT], BF16, name=f"x_{mname}{kt}", tag="v", bufs=20)
            nc.gpsimd.dma_start(t, x_ap[kt * P:kt * P + kp, :])
            xin[(mi, kt)] = t

    proj_t = {}
    sinv_t = {}

    def proj_mm(mi):
        mname, nkt, _, w_ap = mod_meta[mi]
        kp = P if mname != "lidar" else 64
        wtag, wbufs = ("wlid", 1) if mname == "lidar" else ("wproj", 2)
        ws = sb.tile([kp, nkt * BC], BF16, name=f"w_{mname}_s", tag=wtag, bufs=wbufs)
        if nkt > 1:
            dma_split(ws, w_ap)
        else:
            nc.sync.dma_start(ws, w_ap)
        proj = []
        ss_ps = psum.tile([1, NT], F32, name=f"ss_{mname}", tag="pat", bufs=2)
        for ct in range(8):
            ps = psum.tile([P, NT], F32, name=f"pp_{mname}{ct}", tag="pmm", bufs=3)
            for kt in range(nkt):
                mm(ps, ws[:, kt * BC + ct * P: kt * BC + (ct + 1) * P],
                   xin[(mi, kt)], start=(kt == 0), stop=(kt == nkt - 1))
            pt = sb.tile([P, NT], BF16, name=f"proj_{mname}{ct}", tag="v", bufs=20)
            nc.scalar.activation(pt, ps, AF.Identity,
                                 bias=bias_proj[:, mi * 8 + ct: mi * 8 + ct + 1])
            proj.append(pt)
            sq = sb.tile([P, NT], BF16, name=f"sq_{mname}{ct}", tag="sq", bufs=2)
            nc.scalar.activation(sq, pt, AF.Square)
            mm(ss_ps, ones_col_bf, sq, start=(ct == 0), stop=(ct == 7))
        snorm = sb.tile([1, NT], F32, name=f"snorm_{mname}", tag="row", bufs=4)
        nc.scalar.activation(snorm, ss_ps, AF.Sqrt)
        sinv = sb.tile([1, NT], BF16, name=f"sinv_{mname}", tag="row", bufs=4)
        with nc.allow_low_precision(reason="1/norm broadcast as bf16 matmul rhs"):
            nc.vector.reciprocal(sinv, snorm)
        proj_t[mi] = proj
        sinv_t[mi] = sinv

    def norm_mul(mi):
        mname = mod_meta[mi][0]
        bc_ps = psum.tile([P, NT], F32, name=f"bc_{mname}", tag="pmm", bufs=3)
        mm(bc_ps, ones_row_bf, sinv_t[mi])
        proj = proj_t[mi]
        for ct in range(8):
            rt = sb.tile([P, NT], BF16, name=f"f_{mname}{ct}", tag="rdl", bufs=24)
            nc.vector.tensor_mul(rt, proj[ct], bc_ps)
            if ct % 2 == 0:
                f8[mi][ct // 2] = sb.tile([P, 2 * NT], FP8,
                                          name=f"f8_{mname}{ct // 2}",
                                          tag="f8", bufs=12)
            # fp8 copy (x ASCALE) for the QKV GEMM inputs
            nc.vector.tensor_scalar_mul(
                f8[mi][ct // 2][:, (ct % 2) * NT:(ct % 2 + 1) * NT],
                rt, ASCALE)
            # fold the (pair-summed) o-bias into the residual tile
            nc.vector.tensor_scalar_add(
                rt, rt, bias_o[:, mi * 8 + ct: mi * 8 + ct + 1])
            rdl[mi][ct] = rt

    # interleave: projections keep the PE busy while norm chains resolve
    proj_mm(1)          # depth
    proj_mm(0)          # rgb
    norm_mul(1)
    proj_mm(2)          # lidar
    norm_mul(0)
    norm_mul(2)

    # small resident weights (needed from se_finish(0) onward; emitted here so
    # their slow small-line DMAs sit behind the phase-A critical loads)
    w_gate_s = const.tile([P, 24 * 3], BF16, name="w_gate_s")
    nc.gpsimd.dma_start(w_gate_s.rearrange("p (kt n) -> p kt n", kt=24),
                        p["w_gate"].rearrange("(kt p) n -> p kt n", p=P))
    w_se1_s = const.tile([P, 3 * 8 * SEH], BF16, name="w_se1_s")
    nc.gpsimd.dma_start(
        w_se1_s.rearrange("p (m kt n) -> p m kt n", m=3, kt=8),
        p["w_se1"].rearrange("m (kt p) n -> p m kt n", p=P))

    # ---------------- attention ----------------
    def qkv_phase(i):
        qm, km = BLOCKS[i]
        f8q, f8kv = f8[qm], f8[km]

        def w8_slab(ap, label, eng):
            t = sb.tile([P, 8192], FP8, name=f"{label}{i}", tag="wbig", bufs=4)
            eng.dma_start(t, ap)
            return t.rearrange("p (ktp g n) -> p ktp g n", ktp=4, g=2)

        wv = w8_slab(p["w8_v"][i], "wv", nc.sync)
        wk = w8_slab(p["w8_k"][i], "wk", nc.scalar)
        wq = w8_slab(p["w8_q"][i], "wq", nc.sync)

        v_cm = []
        for ct in range(8):
            ps = psum.tile([P, NT], F32, name=f"ps_v{i}_{ct}", tag="pmm", bufs=3)
            for ktp in range(4):
                mm(ps, wv[:, ktp, :, ct * P:(ct + 1) * P],
                   f8kv[ktp].rearrange("p (g n) -> p g n", g=2),
                   start=(ktp == 0), stop=(ktp == 3), perf_mode=DR)
            vt = sb.tile([P, NT], BF16, name=f"v{i}_{ct}", tag="v", bufs=20)
            nc.scalar.activation(vt, ps, AF.Identity,
                                 scale=desc[:, 12 + i:13 + i],
                                 bias=bias_v[:, i * 8 + ct: i * 8 + ct + 1])
            v_cm.append(vt)

        def tok_major(ws, dcol, bseg, label, f8m):
            bbc = sb.tile([P, BC], BF16, name=f"b_{label}{i}", tag="qkbc", bufs=2)
            nc.scalar.dma_start(bbc, p["qk_bias"][bseg])
            tiles = []
            for mt in range(4):
                t = sb.tile([P, BC], BF16, name=f"{label}{i}m{mt}", tag="qk", bufs=9)
                for nt2 in range(2):
                    ps = psum.tile([P, 512], F32, name=f"ps_{label}{i}_{mt}{nt2}",
                                   tag="pmm", bufs=3)
                    for ktp in range(4):
                        mm(ps,
                           f8m[ktp].rearrange("p (g n) -> p g n", g=2)[
                               :, :, mt * P:(mt + 1) * P],
                           ws[:, ktp, :, nt2 * 512:(nt2 + 1) * 512],
                           start=(ktp == 0), stop=(ktp == 3), perf_mode=DR)
                    nc.vector.scalar_tensor_tensor(
                        t[:, nt2 * 512:(nt2 + 1) * 512], ps,
                        desc[:, dcol:dcol + 1],
                        bbc[:, nt2 * 512:(nt2 + 1) * 512],
                        op0=ALU.mult, op1=ALU.add)
                tiles.append(t)
            return tiles

        k_tm = tok_major(wk, 6 + i, 2 * i + 1, "k", f8kv)
        q_tm = tok_major(wq, i, 2 * i, "q", f8q)

        # scoresT[k, h] partials: contract over the 512 local positions
        stage = sb.tile([P, HEADS * P], BF16, name=f"sstage{i}", tag="sc", bufs=3)
        for h in range(HEADS):
            ps = psum.tile([P, P], F32, name=f"ps_sc{i}_{h}", tag="psc", bufs=2)
            for mt in range(4):
                mm(ps, k_tm[mt][:, h * HD:(h + 1) * HD],
                   q_tm[mt][:, h * HD:(h + 1) * HD],
                   start=(mt == 0), stop=(mt == 3))
            nc.vector.tensor_copy(stage[:, h * P:(h + 1) * P], ps)

        cc_in = dram.tile([P, HEADS * P], BF16, name=f"ccin{i}", tag=f"ccin{i}")
        cc_out = dram.tile([P, HEADS * P], BF16, name=f"ccout{i}", tag=f"ccout{i}")
        nc.gpsimd.dma_start(cc_in, stage)
        nc.gpsimd.collective_compute(
            "AllReduce", ALU.add, replica_groups=RG_PAIR,
            ins=[cc_in.opt()], outs=[cc_out.opt()])
        gather = sb.tile([P, HEADS * P], BF16, name=f"sgather{i}", tag="sc", bufs=3)
        nc.gpsimd.dma_start(gather, cc_out)
        return v_cm, gather

    def attn_phase(i, v_cm, gather):
        """softmax (no max-subtract; |logits| < 0.02) + attn @ V -> fp8 pairs."""
        ao8 = []
        for h in range(HEADS):
            ex = sb.tile([P, P], BF16, name=f"ex{i}_{h}", tag="ex", bufs=4)
            nc.scalar.activation(ex, gather[:, h * P:(h + 1) * P], AF.Exp,
                                 scale=SCALE)
            ps = psum.tile([P, NT], F32, name=f"ps_at{i}_{h}", tag="pat", bufs=2)
            mm(ps, ex, v_cm[h])
            cs = psum.tile([P, 1], F32, name=f"cs{i}_{h}", tag="psc", bufs=2)
            mm(cs, ex, inv16_col_bf)          # sum(exp)/ASCALE
            rec = sb.tile([P, 1], F32, name=f"rec{i}_{h}", tag="rec", bufs=4)
            nc.vector.reciprocal(rec, cs)     # ASCALE / sum(exp)
            if h % 2 == 0:
                t8 = sb.tile([P, 2 * NT], FP8, name=f"ao8_{i}_{h // 2}",
                             tag="ao8", bufs=10)
                ao8.append(t8)
            nc.vector.tensor_scalar_mul(
                ao8[h // 2][:, (h % 2) * NT:(h % 2 + 1) * NT], ps, rec)
        return ao8

    def o_pair(m, ao_even, ao_odd):
        st = sb.tile([P, 8], F32, name=f"se_st{m}", tag=f"se_a{m}", bufs=1)
        i0, i1 = 2 * m, 2 * m + 1
        wo0 = sb.tile([P, 8192], FP8, name=f"wo{i0}", tag="wbig", bufs=4)
        nc.sync.dma_start(wo0, p["w8_o"][i0])
        wo0r = wo0.rearrange("p (ktp g n) -> p ktp g n", ktp=4, g=2)
        wo1 = sb.tile([P, 8192], FP8, name=f"wo{i1}", tag="wbig", bufs=4)
        nc.scalar.dma_start(wo1, p["w8_o"][i1])
        wo1r = wo1.rearrange("p (ktp g n) -> p ktp g n", ktp=4, g=2)
        cross = []
        for ct in range(8):
            ps = psum.tile([P, NT], F32, name=f"ps_o{m}_{ct}", tag="pmm", bufs=3)
            for ktp in range(4):
                mm(ps, wo0r[:, ktp, :, ct * P:(ct + 1) * P],
                   ao_even[ktp].rearrange("p (g n) -> p g n", g=2),
                   start=(ktp == 0), stop=False, perf_mode=DR)
            for ktp in range(4):
                mm(ps, wo1r[:, ktp, :, ct * P:(ct + 1) * P],
                   ao_odd[ktp].rearrange("p (g n) -> p g n", g=2),
                   start=False, stop=(ktp == 3), perf_mode=DR)
            crt = sb.tile([P, NT], BF16, name=f"cross{m}_{ct}", tag="cross", bufs=24)
            # desc * psum + (residual + pair-summed o-bias); accum_out gives
            # the SE global-average-pool sum for free
            nc.vector.scalar_tensor_tensor(
                crt, ps, desc[:, 18 + m:19 + m], rdl[m][ct],
                op0=ALU.mult, op1=ALU.add, accum_out=st[:, ct:ct + 1])
            cross.append(crt)
        # launch the SE pool AllReduce immediately
        se_in = dram.tile([P, 8], F32, name=f"se_ccin{m}", tag=f"se_ccin{m}")
        se_out = dram.tile([P, 8], F32, name=f"se_ccout{m}", tag=f"se_ccout{m}")
        nc.gpsimd.dma_start(se_in, st)
        nc.gpsimd.collective_compute(
            "AllReduce", ALU.add, replica_groups=RG_PAIR,
            ins=[se_in.opt()], outs=[se_out.opt()])
        pf = sb.tile([P, 8], F32, name=f"se_pf{m}", tag=f"se_b{m}", bufs=1)
        nc.gpsimd.dma_start(pf, se_out)
        return cross, pf

    gse = [None] * 3
    wg_eff = sb.tile([P, 24 * 3], BF16, name="wg_eff", tag="wge", bufs=1)

    def se_finish(m, pf):
        """SE MLP for modality m (AllReduce + receive were launched by
        o_pair); folds the SE gate into the gate-conv weights (per-core batch
        is fixed, so the gate is a per-channel scalar)."""
        pb = sb.tile([P, 8], BF16, name=f"se_pb{m}", tag=f"se_c{m}", bufs=1)
        nc.vector.tensor_copy(pb, pf)
        h_ps = psum.tile([SEH, 1], F32, name=f"h_ps{m}", tag="psc", bufs=2)
        for kt in range(8):
            mm(h_ps, w_se1_s[:, (m * 8 + kt) * SEH: (m * 8 + kt + 1) * SEH],
               pb[:, kt:kt + 1], start=(kt == 0), stop=(kt == 7))
        h_sb = sb.tile([SEH, 1], BF16, name=f"h_sb{m}", tag="rec", bufs=4)
        nc.scalar.activation(h_sb, h_ps, AF.Relu, bias=b_se1[:, m:m + 1])
        sew = sb.tile([SEH, BC], BF16, name=f"sew{m}", tag="sew", bufs=2)
        nc.gpsimd.dma_start(sew, p["w_se2"][m])
        gm = sb.tile([P, 8], F32, name=f"gse{m}", tag="gse", bufs=3)
        for ct in range(8):
            g_ps = psum.tile([P, 1], F32, name=f"g_ps{m}_{ct}", tag="psc", bufs=2)
            mm(g_ps, sew[:, ct * P:(ct + 1) * P], h_sb)
            nc.scalar.activation(gm[:, ct:ct + 1], g_ps, AF.Sigmoid,
                                 bias=b_se2[:, m * 8 + ct: m * 8 + ct + 1])
        gse[m] = gm
        for ct in range(8):
            kt = m * 8 + ct
            nc.vector.tensor_scalar_mul(wg_eff[:, kt * 3:(kt + 1) * 3],
                                        w_gate_s[:, kt * 3:(kt + 1) * 3],
                                        gm[:, ct:ct + 1])

    def sparsity_metric(mq):
        sp_ps = psum.tile([1, NT], F32, name=f"sp_ps{mq}", tag="pat", bufs=2)
        for ct in range(8):
            msk = sb.tile([P, NT], BF16, name=f"msk{mq}_{ct}", tag="sq", bufs=2)
            # rdl == bias_o  <=>  normalized feature == 0
            nc.vector.tensor_scalar(
                msk, rdl[mq][ct], bias_o[:, mq * 8 + ct: mq * 8 + ct + 1],
                None, op0=ALU.is_equal)
            mm(sp_ps, ones_col_bf, msk, start=(ct == 0), stop=(ct == 7))
        sp_row = sb.tile([1, NT], BF16, name=f"sp_row{mq}", tag="row", bufs=4)
        nc.scalar.activation(sp_row, sp_ps, AF.Copy, scale=1.0 / 1024.0)
        return sp_row

    # pipelined blocks: AllReduce(i) overlaps the next QKV phase; SE chains
    # are emitted after independent PE work so their AllReduce never stalls
    # the in-order PE queue
    v0, g0 = qkv_phase(0)
    v1, g1 = qkv_phase(1)
    ao0 = attn_phase(0, v0, g0)
    v2, g2 = qkv_phase(2)
    ao1 = attn_phase(1, v1, g1)
    cross_rgb, seo0 = o_pair(0, ao0, ao1)
    v3, g3 = qkv_phase(3)
    ao2 = attn_phase(2, v2, g2)
    se_finish(0, seo0)
    v4, g4 = qkv_phase(4)
    ao3 = attn_phase(3, v3, g3)
    cross_dep, seo1 = o_pair(1, ao2, ao3)
    v5, g5 = qkv_phase(5)
    ao4 = attn_phase(4, v4, g4)

    # quality metrics (from the rdl tiles = normalized features + o-bias;
    # the o-bias shift is compensated exactly) - fill the SE wait
    rsum_ps = psum.tile([1, NT], F32, name="rsum_ps", tag="pat", bufs=2)
    for ct in range(8):
        mm(rsum_ps, ones_col_bf, rdl[0][ct], start=(ct == 0), stop=(ct == 7))
    # var = 1/1023 - (rsum' - C)^2/(1024*1023)  with C = sum(bias_o_rgb)
    xr = []
    var_row = sb.tile([1, NT], BF16, name="var_row", tag="row", bufs=4)
    nc.scalar.activation(var_row, rsum_ps, AF.Square,
                         scale=float(1.0 / np.sqrt(1024.0 * 1023.0)),
                         bias=desc[0:1, 21:22])
    nc.vector.tensor_scalar(var_row, var_row, -1.0, 1.0 / 1023.0,
                            op0=ALU.mult, op1=ALU.add)
    xr.append(var_row)
    xr.append(sparsity_metric(1))

    se_finish(1, seo1)
    crosses_rd = [cross_rgb, cross_dep]

    # rgb/depth contributions to the gate conv - filler while AR(5) flies
    gt_ps = psum.tile([3, NT], F32, name="gt_ps", tag="prow", bufs=1)
    for m in range(2):
        for ct in range(8):
            kt = m * 8 + ct
            mm(gt_ps, wg_eff[:, kt * 3:(kt + 1) * 3], crosses_rd[m][ct],
               start=(kt == 0), stop=False)

    ao5 = attn_phase(5, v5, g5)
    cross_lid, seo2 = o_pair(2, ao4, ao5)
    xr.append(sparsity_metric(2))      # fills the SE(2) AllReduce wait
    se_finish(2, seo2)
    crosses = [cross_rgb, cross_dep, cross_lid]

    # lidar part + extras of the gate conv; rgb/depth parts were emitted early
    for ct in range(8):
        mm(gt_ps, wg_eff[:, (16 + ct) * 3:(17 + ct) * 3], crosses[2][ct],
           start=False, stop=False)
    for m in range(3):
        mm(gt_ps, w_gate_x[0:1, 3 * m:3 * m + 3], xr[m],
           start=False, stop=(m == 2))
    gates = sb.tile([3, NT], BF16, name="gates", tag="row", bufs=4)
    nc.scalar.activation(gates, gt_ps, AF.Sigmoid, bias=b_gate[:, 0:1])

    # ---------------- Phase E: fused features + fusion conv ----------------
    # fused = cross * se_gate * modality_gate, in place in one pass
    for m in range(3):
        bc_ps = psum.tile([P, NT], F32, name=f"gbc{m}", tag="pmm", bufs=3)
        mm(bc_ps, sel3[:, m * P:(m + 1) * P], gates)
        for ct in range(8):
            nc.vector.scalar_tensor_tensor(
                crosses[m][ct], crosses[m][ct], gse[m][:, ct:ct + 1], bc_ps,
                op0=ALU.mult, op1=ALU.mult)
    fused = [crosses[m][ct] for m in range(3) for ct in range(8)]

    for ct in range(24):
        wf = sb.tile([P, 24 * P], BF16, name=f"wf{ct}", tag="wfuse", bufs=2)
        eng = nc.sync if ct % 2 == 0 else nc.scalar
        eng.dma_start(wf, p["w_fuse"][:, ct * TOTAL:(ct + 1) * TOTAL])
        ps = psum.tile([P, NT], F32, name=f"ps_f{ct}", tag="pmm", bufs=3)
        for kt in range(24):
            mm(ps, wf[:, kt * P:(kt + 1) * P], fused[kt],
               start=(kt == 0), stop=(kt == 23))
        ot = sb.tile([P, NT], F32, name=f"ot{ct}", tag="outb", bufs=2)
        nc.scalar.activation(ot, ps, AF.Identity, bias=bias_fuse[:, ct: ct + 1])
        nc.gpsimd.dma_start(p["out"][ct * P:(ct + 1) * P, :], ot)


def _build():
    nc = bacc.Bacc("TRN2", target_bir_lowering=False, debug=False,
                   num_devices=NCORES)
    params = _declare_params(nc)
    with tile.TileContext(nc) as tc, ExitStack() as ctx:
        _emit(nc, tc, params, ctx)
    nc.compile()
    return nc


def _fp8_slab(wT, s):
    """[1024, 1024] (already [Cin, Cout]) -> [128, 8192] fp8 DoubleRow image."""
    q8 = (wT * s).astype(e4m3)
    return np.ascontiguousarray(
        q8.reshape(4, 2, P, BC).transpose(2, 0, 1, 3).reshape(P, 8192))


def _pow2_scale(w):
    """largest power of 2 with max|w|*s <= 224"""
    m = float(np.abs(w).max())
    return 2.0 ** np.floor(np.log2(224.0 / m))


def _prep_static(inputs):
    """Host-side weight prep shared by all cores."""
    f32 = np.float32

    def colpack(b):  # [1024] -> [128, 8] (col j = channel tile j)
        return np.ascontiguousarray(b.reshape(8, P).T.astype(f32))

    def packslab(wT, kt):   # [kt*128, n] -> [128, kt*n] SBUF image
        n = wT.shape[1]
        return np.ascontiguousarray(
            wT.reshape(kt, -1, n).transpose(1, 0, 2).reshape(-1, kt * n))

    s = {}
    s["w_rgb"] = packslab(inputs["proj_rgb_w"].T.astype(bf16), 4)
    s["w_depth"] = packslab(inputs["proj_depth_w"].T.astype(bf16), 2)
    s["w_lidar"] = np.ascontiguousarray(inputs["proj_lidar_w"].T.astype(bf16))

    desc = np.zeros((P, 24), f32)
    for nm, base in (("q", 0), ("k", 6), ("v", 12)):
        w = inputs[f"attn_{nm}_w"]
        slabs = np.empty((6, P, 8192), e4m3)
        for i in range(6):
            wT = np.ascontiguousarray(w[i].T)
            sc = _pow2_scale(wT)
            slabs[i] = _fp8_slab(wT, sc)
            desc[:, base + i] = 1.0 / (sc * ASCALE)
        s[f"w8_{nm}"] = slabs
    wo = inputs["attn_o_w"]
    slabs = np.empty((6, P, 8192), e4m3)
    for m in range(3):
        sc = min(_pow2_scale(wo[2 * m].T), _pow2_scale(wo[2 * m + 1].T))
        slabs[2 * m] = _fp8_slab(np.ascontiguousarray(wo[2 * m].T), sc)
        slabs[2 * m + 1] = _fp8_slab(np.ascontiguousarray(wo[2 * m + 1].T), sc)
        desc[:, 18 + m] = 1.0 / (sc * ASCALE)
    s["w8_o"] = slabs
    # var-metric bias: -sum(bias_o_rgb)/sqrt(1024*1023), from the
    # bf16-rounded bias actually folded into the rdl tiles
    bo_rgb = (inputs["attn_o_b"][0] + inputs["attn_o_b"][1]) \
        .astype(bf16).astype(np.float64)
    desc[0, 21] = float(-bo_rgb.sum() / np.sqrt(1024.0 * 1023.0))
    s["desc"] = desc

    qk = np.empty((12, BC), f32)
    for i in range(6):
        qk[2 * i] = inputs["attn_q_b"][i]
        qk[2 * i + 1] = inputs["attn_k_b"][i]
    s["qk_bias"] = np.ascontiguousarray(
        np.broadcast_to(qk[:, None, :], (12, P, BC))).astype(bf16)
    s["bias_proj"] = np.concatenate(
        [colpack(inputs["proj_rgb_b"]), colpack(inputs["proj_depth_b"]),
         colpack(inputs["proj_lidar_b"])], axis=1)
    s["bias_v"] = np.concatenate(
        [colpack(inputs["attn_v_b"][i]) for i in range(6)], axis=1)
    # round the pair-summed o-bias through bf16 so that the on-device
    # is_equal sparsity test (rdl == bias_o) is exact for true zeros
    s["bias_o"] = np.concatenate(
        [colpack((inputs["attn_o_b"][2 * m] + inputs["attn_o_b"][2 * m + 1])
                 .astype(bf16).astype(np.float32))
         for m in range(3)], axis=1)
    s["w_se1"] = np.ascontiguousarray(
        (inputs["se_w1"] / 1024.0).transpose(0, 2, 1)).astype(bf16)
    s["b_se1"] = np.ascontiguousarray(inputs["se_b1"].T.astype(f32))
    s["w_se2"] = np.ascontiguousarray(inputs["se_w2"].transpose(0, 2, 1)).astype(bf16)
    s["b_se2"] = np.concatenate(
        [colpack(inputs["se_b2"][m]) for m in range(3)], axis=1)
    s["w_gate"] = np.ascontiguousarray(inputs["gate_w"][:, :TOTAL].T).astype(bf16)
    s["w_gate_x"] = np.ascontiguousarray(
        inputs["gate_w"][:, TOTAL:].T.astype(f32)).reshape(1, 9).astype(bf16)
    s["b_gate"] = inputs["gate_b"].reshape(3, 1).astype(f32)
    # fusion slab image: [p, ct, kt, n] contiguous per output tile ct
    wfT = inputs["fusion_w"].T.astype(bf16)   # [k, n] = [3072, 3072]
    s["w_fuse"] = np.ascontiguousarray(
        wfT.reshape(24, P, 24, P).transpose(1, 2, 0, 3).reshape(P, 24 * TOTAL))
    s["bias_fuse"] = np.concatenate(
        [colpack(inputs["fusion_b"][ct * BC:(ct + 1) * BC]) for ct in range(3)],
        axis=1)
    sel = np.zeros((3, 3 * P), bf16)
    for m in range(3):
        sel[m, m * P:(m + 1) * P] = 1.0
    s["sel3"] = sel
    return s


def kernel(**inputs):
    global _compiled, LAST_RESULTS
    if _compiled is None:
        _compiled = _build()
    nc = _compiled

    static = _prep_static(inputs)

    def shard(x, c):  # core c: batch c//2, position half c%2, channel-major
        b, hf = c // 2, c % 2
        C = x.shape[1]
        return np.ascontiguousarray(
            x.reshape(B, C, HW)[b, :, hf * NT:(hf + 1) * NT]).astype(bf16)

    in_maps = []
    for c in range(NCORES):
        m = dict(static)
        m["x_rgb"] = shard(inputs["rgb_features"], c)
        m["x_depth"] = shard(inputs["depth_features"], c)
        m["x_lidar"] = shard(inputs["lidar_features"], c)
        in_maps.append(m)

    res = run_bass_kernel_spmd(nc, in_maps, core_ids=list(range(NCORES)),
                               trace=TRACE)
    LAST_RESULTS = res

    full = np.empty((B, TOTAL, HW), np.float32)
    for c in range(NCORES):
        b, hf = c // 2, c % 2
        full[b, :, hf * NT:(hf + 1) * NT] = res.results[c]["out"]
    return full.reshape(B, TOTAL, 32, 32)


# revision 24
# speedup vs baseline: 1.1153x; 1.0047x over previous
"""Trainium2 distributed Bass kernel for AdaptiveGatedAttentionFusion.

Sharding: each of the 8 cores owns (batch b = core//2, half hf = core%2 of the
1024 spatial positions) -> 512 tokens per core. All weights replicated.
Activations channel-major [C, pos] in SBUF.

v2: the 24 heavy QKV/O 1024x1024 GEMMs run in fp8-e4m3 DoubleRow mode
(256-deep contraction per matmul, ~1.7x TensorE streaming) with fp32 PSUM
accumulation and data-driven descale factors.  Normalized features are
provably bounded (per-token L2 norm == 1), so fixed fp8 scales cannot
overflow.  Projections / fusion / scores / attn@V stay bf16 (fp8 there blows
the error budget - measured by simulation).

Cross-core communication (pairwise, cores (2b, 2b+1) share batch b):
  - one AllReduce per attention block for the head_dim x head_dim score
    matrices (spatial contraction split across the pair), overlapped with
    the next block's QKV projections.
  - one tiny AllReduce per modality for the SE global-average-pool sums.
"""

from contextlib import ExitStack

import numpy as np
import ml_dtypes

import concourse.bacc as bacc
import concourse.bass as bass
import concourse.mybir as mybir
import concourse.tile as tile
from concourse.bass_utils import run_bass_kernel_spmd

F32 = mybir.dt.float32
BF16 = mybir.dt.bfloat16
FP8 = mybir.dt.float8e4
AF = mybir.ActivationFunctionType
ALU = mybir.AluOpType
DR = mybir.MatmulPerfMode.DoubleRow

NCORES = 8
B = 4
HW = 1024
NT = 512                    # tokens per core (half the positions of one batch)
BC = 1024
HEADS = 8
HD = 128
SEH = 64                    # BC / reduction(16)
TOTAL = 3 * BC
SCALE = float(HD) ** -0.5
P = 128
ASCALE = 16.0               # fp8 activation scale (|act| <= ~4 everywhere)

RGB_C, DEPTH_C, LIDAR_C = 512, 256, 64
# attention block wiring: (q_modality, kv_modality); 0=rgb 1=depth 2=lidar
BLOCKS = [(0, 1), (0, 2), (1, 0), (1, 2), (2, 0), (2, 1)]
# pairwise replica groups: cores (2b, 2b+1) hold the two halves of batch b
RG_PAIR = [[0, 1], [2, 3], [4, 5], [6, 7]]

bf16 = ml_dtypes.bfloat16
e4m3 = ml_dtypes.float8_e4m3   # TRN FP8_EXP4-compatible (bias 7, max 240)

_compiled = None
LAST_RESULTS = None       # BassKernelResults of last run (for test harness)
TRACE = False


def _declare_params(nc):
    p = {}

    def inp(name, shape, dt):
        p[name] = nc.dram_tensor(name, list(shape), dt, kind="ExternalInput").ap()

    # per-core activation shard, channel-major [C, NT]
    inp("x_rgb", (RGB_C, NT), BF16)
    inp("x_depth", (DEPTH_C, NT), BF16)
    inp("x_lidar", (LIDAR_C, NT), BF16)
    # projection weight slabs, prepacked SBUF image [p, kt, n] (contiguous)
    inp("w_rgb", (P, 4 * BC), BF16)
    inp("w_depth", (P, 2 * BC), BF16)
    inp("w_lidar", (64, BC), BF16)
    # attention weights: fp8 DoubleRow slabs, SBUF image layout
    # slab[p, ktp*2048 + g*1024 + n] = (w.T)[ktp*256 + g*128 + p, n] * s_w
    inp("w8_q", (6, P, 8192), FP8)
    inp("w8_k", (6, P, 8192), FP8)
    inp("w8_v", (6, P, 8192), FP8)
    inp("w8_o", (6, P, 8192), FP8)
    # q/k biases pre-broadcast to all 128 partitions (row 2i = q_b[i], 2i+1 = k_b[i])
    inp("qk_bias", (12, P, BC), BF16)
    # per-partition bias columns, f32, col j = channel tile j
    inp("bias_proj", (P, 24), F32)     # rgb 0:8, depth 8:16, lidar 16:24
    inp("bias_v", (P, 48), F32)        # block i cols 8i:8i+8
    inp("bias_o", (P, 24), F32)        # modality m cols 8m:8m+8 (pair-summed)
    # fp8 descale factors (per-partition broadcast columns):
    # cols 0-5 q[i], 6-11 k[i], 12-17 v[i], 18-20 o[m], col 21 row0 = var bias
    inp("desc", (P, 24), F32)
    # SE
    inp("w_se1", (3, BC, SEH), BF16)   # (se_w1/1024).T per modality
    inp("b_se1", (SEH, 3), F32)
    inp("w_se2", (3, SEH, BC), BF16)   # se_w2.T
    inp("b_se2", (P, 24), F32)         # modality m cols 8m:8m+8
    # gate
    inp("w_gate", (TOTAL, 3), BF16)    # gate_w[:, :3072].T
    inp("w_gate_x", (1, 9), BF16)      # seg m = gate_w[:, 3072+m] (var, sp_d, sp_l)
    inp("b_gate", (3, 1), F32)
    # fusion weight slabs, prepacked [p, ct, kt, n] (contiguous per ct)
    inp("w_fuse", (P, 24 * TOTAL), BF16)
    inp("bias_fuse", (P, 24), F32)
    # one-hot selector: sel3[:, m*128:(m+1)*128] broadcasts row m of a [3, N]
    # rhs to all 128 output partitions
    inp("sel3", (3, 3 * P), BF16)

    p["out"] = nc.dram_tensor("out", [TOTAL, NT], F32, kind="ExternalOutput").ap()
    return p


def _emit(nc, tc, p, ctx):
    mm = nc.tensor.matmul

    dram = ctx.enter_context(tc.tile_pool(name="dram", bufs=1, space="DRAM"))
    psum = ctx.enter_context(tc.tile_pool(name="psum", bufs=1, space="PSUM"))
    const = ctx.enter_context(tc.tile_pool(name="const", bufs=1))
    sb = ctx.enter_context(tc.tile_pool(name="sb", bufs=1))

    # ---- constants ----
    ones_col_bf = const.tile([P, 1], BF16, name="ones_col_bf")
    nc.vector.memset(ones_col_bf, 1.0)
    inv16_col_bf = const.tile([P, 1], BF16, name="inv16_col_bf")
    nc.vector.memset(inv16_col_bf, 1.0 / ASCALE)
    ones_row_bf = const.tile([1, P], BF16, name="ones_row_bf")
    nc.vector.memset(ones_row_bf, 1.0)

    def load_const(name, ap):
        t = const.tile(list(ap.shape), ap.dtype, name=name)
        nc.gpsimd.dma_start(t, ap)
        return t

    # phase-A-critical consts first on the gpsimd queue
    bias_proj = load_const("bias_proj", p["bias_proj"])
    bias_o = load_const("bias_o", p["bias_o"])

    def dma_split(dst, src):
        """prepacked slab load, halves on separate queues"""
        h = src.shape[-1] // 2
        nc.sync.dma_start(dst[:, :h], src[:, :h])
        nc.scalar.dma_start(dst[:, h:], src[:, h:])

    # warm up the collectives path (first AllReduce pays ~10us extra)
    cw_in = dram.tile([1, 16], F32, name="ccwarm_in", tag="ccwarm_in")
    cw_out = dram.tile([1, 16], F32, name="ccwarm_out", tag="ccwarm_out")
    warm_z = const.tile([1, 16], F32, name="warm_z")
    nc.vector.memset(warm_z, 0.0)
    nc.gpsimd.dma_start(cw_in, warm_z)
    nc.gpsimd.collective_compute(
        "AllReduce", ALU.add, replica_groups=RG_PAIR,
        ins=[cw_in.opt()], outs=[cw_out.opt()])

    # ---------------- Phase A: projections + L2 normalize ----------------
    # rdl[m][ct] holds (normalized features + o-bias) = residual tiles.
    # f8[m][pair] holds fp8 copies (x ASCALE) of the normalized features.
    rdl = [[None] * 8 for _ in range(3)]
    f8 = [[None] * 4 for _ in range(3)]
    mod_meta = [("rgb", 4, p["x_rgb"], p["w_rgb"]),
                ("depth", 2, p["x_depth"], p["w_depth"]),
                ("lidar", 1, p["x_lidar"], p["w_lidar"])]

    xin = {}
    for mi in (1, 2, 0):
        mname, nkt, x_ap, _ = mod_meta[mi]
        kp = P if mname != "lidar" else 64
        for kt in range(nkt):
            t = sb.tile([kp, NT], BF16, name=f"x_{mname}{kt}", tag="v", bufs=20)
            nc.gpsimd.dma_start(t, x_ap[kt * P:kt * P + kp, :])
            xin[(mi, kt)] = t

    desc = load_const("desc", p["desc"])
    bias_v = load_const("bias_v", p["bias_v"])
    b_se1 = load_const("b_se1", p["b_se1"])
    b_se2 = load_const("b_se2", p["b_se2"])
    b_gate = load_const("b_gate", p["b_gate"])
    bias_fuse = load_const("bias_fuse", p["bias_fuse"])
    w_gate_x = load_const("w_gate_x", p["w_gate_x"])
    sel3 = load_const("sel3", p["sel3"])

    proj_t = {}
    sinv_t = {}

    def proj_mm(mi):
        mname, nkt, _, w_ap = mod_meta[mi]
        kp = P if mname != "lidar" else 64
        wtag, wbufs = ("wlid", 1) if mname == "lidar" else ("wproj", 2)
        ws = sb.tile([kp, nkt * BC], BF16, name=f"w_{mname}_s", tag=wtag, bufs=wbufs)
        if nkt > 2:
            dma_split(ws, w_ap)
        else:
            nc.sync.dma_start(ws, w_ap)
        proj = []
        ss_ps = psum.tile([1, NT], F32, name=f"ss_{mname}", tag="pat", bufs=2)
        for ct in range(8):
            ps = psum.tile([P, NT], F32, name=f"pp_{mname}{ct}", tag="pmm", bufs=3)
            for kt in range(nkt):
                mm(ps, ws[:, kt * BC + ct * P: kt * BC + (ct + 1) * P],
                   xin[(mi, kt)], start=(kt == 0), stop=(kt == nkt - 1))
            pt = sb.tile([P, NT], BF16, name=f"proj_{mname}{ct}", tag="v", bufs=20)
            nc.scalar.activation(pt, ps, AF.Identity,
                                 bias=bias_proj[:, mi * 8 + ct: mi * 8 + ct + 1])
            proj.append(pt)
            sq = sb.tile([P, NT], BF16, name=f"sq_{mname}{ct}", tag="sq", bufs=2)
            nc.vector.tensor_mul(sq, pt, pt)
            mm(ss_ps, ones_col_bf, sq, start=(ct == 0), stop=(ct == 7))
        snorm = sb.tile([1, NT], F32, name=f"snorm_{mname}", tag="row", bufs=4)
        nc.scalar.activation(snorm, ss_ps, AF.Sqrt)
        sinv = sb.tile([1, NT], BF16, name=f"sinv_{mname}", tag="row", bufs=4)
        with nc.allow_low_precision(reason="1/norm broadcast as bf16 matmul rhs"):
            nc.vector.reciprocal(sinv, snorm)
        proj_t[mi] = proj
        sinv_t[mi] = sinv

    def norm_mul(mi):
        mname = mod_meta[mi][0]
        bc_ps = psum.tile([P, NT], F32, name=f"bc_{mname}", tag="pmm", bufs=3)
        mm(bc_ps, ones_row_bf, sinv_t[mi])
        proj = proj_t[mi]
        for ct in range(8):
            rt = sb.tile([P, NT], BF16, name=f"f_{mname}{ct}", tag="rdl", bufs=24)
            nc.vector.tensor_mul(rt, proj[ct], bc_ps)
            if ct % 2 == 0:
                f8[mi][ct // 2] = sb.tile([P, 2 * NT], FP8,
                                          name=f"f8_{mname}{ct // 2}",
                                          tag="f8", bufs=12)
            # fp8 copy (x ASCALE) for the QKV GEMM inputs
            nc.vector.tensor_scalar_mul(
                f8[mi][ct // 2][:, (ct % 2) * NT:(ct % 2 + 1) * NT],
                rt, ASCALE)
            # fold the (pair-summed) o-bias into the residual tile
            nc.vector.tensor_scalar_add(
                rt, rt, bias_o[:, mi * 8 + ct: mi * 8 + ct + 1])
            rdl[mi][ct] = rt

    # interleave: projections keep the PE busy while norm chains resolve
    proj_mm(1)          # depth
    proj_mm(2)          # lidar (tiny slab, keeps PE fed while rgb slab lands)
    norm_mul(1)
    proj_mm(0)          # rgb
    norm_mul(2)
    norm_mul(0)

    # small resident weights (needed from se_finish(0) onward; emitted here so
    # their slow small-line DMAs sit behind the phase-A critical loads)
    w_gate_s = const.tile([P, 24 * 3], BF16, name="w_gate_s")
    nc.gpsimd.dma_start(w_gate_s.rearrange("p (kt n) -> p kt n", kt=24),
                        p["w_gate"].rearrange("(kt p) n -> p kt n", p=P))
    w_se1_s = const.tile([P, 3 * 8 * SEH], BF16, name="w_se1_s")
    nc.gpsimd.dma_start(
        w_se1_s.rearrange("p (m kt n) -> p m kt n", m=3, kt=8),
        p["w_se1"].rearrange("m (kt p) n -> p m kt n", p=P))

    # ---------------- attention ----------------
    def qkv_phase(i):
        qm, km = BLOCKS[i]
        f8q, f8kv = f8[qm], f8[km]

        def w8_slab(ap, label, eng):
            t = sb.tile([P, 8192], FP8, name=f"{label}{i}", tag="wbig", bufs=4)
            eng.dma_start(t, ap)
            return t.rearrange("p (ktp g n) -> p ktp g n", ktp=4, g=2)

        wv = w8_slab(p["w8_v"][i], "wv", nc.sync)
        wk = w8_slab(p["w8_k"][i], "wk", nc.scalar)
        wq = w8_slab(p["w8_q"][i], "wq", nc.sync)

        v_cm = []
        for ct in range(8):
            ps = psum.tile([P, NT], F32, name=f"ps_v{i}_{ct}", tag="pmm", bufs=3)
            for ktp in range(4):
                mm(ps, wv[:, ktp, :, ct * P:(ct + 1) * P],
                   f8kv[ktp].rearrange("p (g n) -> p g n", g=2),
                   start=(ktp == 0), stop=(ktp == 3), perf_mode=DR)
            vt = sb.tile([P, NT], BF16, name=f"v{i}_{ct}", tag="v", bufs=20)
            nc.scalar.activation(vt, ps, AF.Identity,
                                 scale=desc[:, 12 + i:13 + i],
                                 bias=bias_v[:, i * 8 + ct: i * 8 + ct + 1])
            v_cm.append(vt)

        def tok_major(ws, dcol, bseg, label, f8m):
            bbc = sb.tile([P, BC], BF16, name=f"b_{label}{i}", tag="qkbc", bufs=2)
            nc.scalar.dma_start(bbc, p["qk_bias"][bseg])
            tiles = []
            for mt in range(4):
                t = sb.tile([P, BC], BF16, name=f"{label}{i}m{mt}", tag="qk", bufs=9)
                for nt2 in range(2):
                    ps = psum.tile([P, 512], F32, name=f"ps_{label}{i}_{mt}{nt2}",
                                   tag="pmm", bufs=3)
                    for ktp in range(4):
                        mm(ps,
                           f8m[ktp].rearrange("p (g n) -> p g n", g=2)[
                               :, :, mt * P:(mt + 1) * P],
                           ws[:, ktp, :, nt2 * 512:(nt2 + 1) * 512],
                           start=(ktp == 0), stop=(ktp == 3), perf_mode=DR)
                    nc.vector.scalar_tensor_tensor(
                        t[:, nt2 * 512:(nt2 + 1) * 512], ps,
                        desc[:, dcol:dcol + 1],
                        bbc[:, nt2 * 512:(nt2 + 1) * 512],
                        op0=ALU.mult, op1=ALU.add)
                tiles.append(t)
            return tiles

        k_tm = tok_major(wk, 6 + i, 2 * i + 1, "k", f8kv)
        q_tm = tok_major(wq, i, 2 * i, "q", f8q)

        # scoresT[k, h] partials: contract over the 512 local positions
        stage = sb.tile([P, HEADS * P], BF16, name=f"sstage{i}", tag="sc", bufs=3)
        for h in range(HEADS):
            ps = psum.tile([P, P], F32, name=f"ps_sc{i}_{h}", tag="psc", bufs=2)
            for mt in range(4):
                mm(ps, k_tm[mt][:, h * HD:(h + 1) * HD],
                   q_tm[mt][:, h * HD:(h + 1) * HD],
                   start=(mt == 0), stop=(mt == 3))
            nc.vector.tensor_copy(stage[:, h * P:(h + 1) * P], ps)

        cc_in = dram.tile([P, HEADS * P], BF16, name=f"ccin{i}", tag=f"ccin{i}")
        cc_out = dram.tile([P, HEADS * P], BF16, name=f"ccout{i}", tag=f"ccout{i}")
        nc.gpsimd.dma_start(cc_in, stage)
        nc.gpsimd.collective_compute(
            "AllReduce", ALU.add, replica_groups=RG_PAIR,
            ins=[cc_in.opt()], outs=[cc_out.opt()])
        gather = sb.tile([P, HEADS * P], BF16, name=f"sgather{i}", tag="sc", bufs=3)
        nc.gpsimd.dma_start(gather, cc_out)
        return v_cm, gather

    def attn_phase(i, v_cm, gather):
        """softmax (no max-subtract; |logits| < 0.02) + attn @ V -> fp8 pairs."""
        ao8 = []
        for h in range(HEADS):
            ex = sb.tile([P, P], BF16, name=f"ex{i}_{h}", tag="ex", bufs=4)
            nc.scalar.activation(ex, gather[:, h * P:(h + 1) * P], AF.Exp,
                                 scale=SCALE)
            ps = psum.tile([P, NT], F32, name=f"ps_at{i}_{h}", tag="pat", bufs=2)
            mm(ps, ex, v_cm[h])
            cs = psum.tile([P, 1], F32, name=f"cs{i}_{h}", tag="psc", bufs=2)
            mm(cs, ex, inv16_col_bf)          # sum(exp)/ASCALE
            rec = sb.tile([P, 1], F32, name=f"rec{i}_{h}", tag="rec", bufs=4)
            nc.vector.reciprocal(rec, cs)     # ASCALE / sum(exp)
            if h % 2 == 0:
                t8 = sb.tile([P, 2 * NT], FP8, name=f"ao8_{i}_{h // 2}",
                             tag="ao8", bufs=10)
                ao8.append(t8)
            nc.vector.tensor_scalar_mul(
                ao8[h // 2][:, (h % 2) * NT:(h % 2 + 1) * NT], ps, rec)
        return ao8

    def o_pair(m, ao_even, ao_odd):
        st = sb.tile([P, 8], F32, name=f"se_st{m}", tag=f"se_a{m}", bufs=1)
        i0, i1 = 2 * m, 2 * m + 1
        wo0 = sb.tile([P, 8192], FP8, name=f"wo{i0}", tag="wbig", bufs=4)
        nc.sync.dma_start(wo0, p["w8_o"][i0])
        wo0r = wo0.rearrange("p (ktp g n) -> p ktp g n", ktp=4, g=2)
        wo1 = sb.tile([P, 8192], FP8, name=f"wo{i1}", tag="wbig", bufs=4)
        nc.scalar.dma_start(wo1, p["w8_o"][i1])
        wo1r = wo1.rearrange("p (ktp g n) -> p ktp g n", ktp=4, g=2)
        cross = []
        for ct in range(8):
            ps = psum.tile([P, NT], F32, name=f"ps_o{m}_{ct}", tag="pmm", bufs=3)
            for ktp in range(4):
                mm(ps, wo0r[:, ktp, :, ct * P:(ct + 1) * P],
                   ao_even[ktp].rearrange("p (g n) -> p g n", g=2),
                   start=(ktp == 0), stop=False, perf_mode=DR)
            for ktp in range(4):
                mm(ps, wo1r[:, ktp, :, ct * P:(ct + 1) * P],
                   ao_odd[ktp].rearrange("p (g n) -> p g n", g=2),
                   start=False, stop=(ktp == 3), perf_mode=DR)
            crt = sb.tile([P, NT], BF16, name=f"cross{m}_{ct}", tag="cross", bufs=24)
            # desc * psum + (residual + pair-summed o-bias); accum_out gives
            # the SE global-average-pool sum for free
            nc.vector.scalar_tensor_tensor(
                crt, ps, desc[:, 18 + m:19 + m], rdl[m][ct],
                op0=ALU.mult, op1=ALU.add, accum_out=st[:, ct:ct + 1])
            cross.append(crt)
        # launch the SE pool AllReduce immediately
        se_in = dram.tile([P, 8], F32, name=f"se_ccin{m}", tag=f"se_ccin{m}")
        se_out = dram.tile([P, 8], F32, name=f"se_ccout{m}", tag=f"se_ccout{m}")
        nc.gpsimd.dma_start(se_in, st)
        nc.gpsimd.collective_compute(
            "AllReduce", ALU.add, replica_groups=RG_PAIR,
            ins=[se_in.opt()], outs=[se_out.opt()])
        pf = sb.tile([P, 8], F32, name=f"se_pf{m}", tag=f"se_b{m}", bufs=1)
        nc.gpsimd.dma_start(pf, se_out)
        return cross, pf

    gse = [None] * 3
    wg_eff = sb.tile([P, 24 * 3], BF16, name="wg_eff", tag="wge", bufs=1)

    def se_finish(m, pf):
        """SE MLP for modality m (AllReduce + receive were launched by
        o_pair); folds the SE gate into the gate-conv weights (per-core batch
        is fixed, so the gate is a per-channel scalar)."""
        pb = sb.tile([P, 8], BF16, name=f"se_pb{m}", tag=f"se_c{m}", bufs=1)
        nc.gpsimd.tensor_copy(pb, pf)
        h_ps = psum.tile([SEH, 1], F32, name=f"h_ps{m}", tag="psc", bufs=2)
        for kt in range(8):
            mm(h_ps, w_se1_s[:, (m * 8 + kt) * SEH: (m * 8 + kt + 1) * SEH],
               pb[:, kt:kt + 1], start=(kt == 0), stop=(kt == 7))
        h_sb = sb.tile([SEH, 1], BF16, name=f"h_sb{m}", tag="rec", bufs=4)
        nc.scalar.activation(h_sb, h_ps, AF.Relu, bias=b_se1[:, m:m + 1])
        sew = sb.tile([SEH, BC], BF16, name=f"sew{m}", tag="sew", bufs=2)
        nc.gpsimd.dma_start(sew, p["w_se2"][m])
        gm = sb.tile([P, 8], F32, name=f"gse{m}", tag="gse", bufs=3)
        for ct in range(8):
            g_ps = psum.tile([P, 1], F32, name=f"g_ps{m}_{ct}", tag="psc", bufs=2)
            mm(g_ps, sew[:, ct * P:(ct + 1) * P], h_sb)
            nc.scalar.activation(gm[:, ct:ct + 1], g_ps, AF.Sigmoid,
                                 bias=b_se2[:, m * 8 + ct: m * 8 + ct + 1])
        gse[m] = gm
        for ct in range(8):
            kt = m * 8 + ct
            nc.gpsimd.tensor_scalar_mul(wg_eff[:, kt * 3:(kt + 1) * 3],
                                        w_gate_s[:, kt * 3:(kt + 1) * 3],
                                        gm[:, ct:ct + 1])

    def sparsity_metric(mq):
        sp_ps = psum.tile([1, NT], F32, name=f"sp_ps{mq}", tag="pat", bufs=2)
        for ct in range(8):
            msk = sb.tile([P, NT], BF16, name=f"msk{mq}_{ct}", tag="sq", bufs=2)
            # rdl == bias_o  <=>  normalized feature == 0
            nc.vector.tensor_scalar(
                msk, rdl[mq][ct], bias_o[:, mq * 8 + ct: mq * 8 + ct + 1],
                None, op0=ALU.is_equal)
            mm(sp_ps, ones_col_bf, msk, start=(ct == 0), stop=(ct == 7))
        sp_row = sb.tile([1, NT], BF16, name=f"sp_row{mq}", tag="row", bufs=4)
        nc.scalar.activation(sp_row, sp_ps, AF.Copy, scale=1.0 / 1024.0)
        return sp_row

    # pipelined blocks: AllReduce(i) overlaps the next QKV phase; SE chains
    # are emitted after independent PE work so their AllReduce never stalls
    # the in-order PE queue
    v0, g0 = qkv_phase(0)
    v1, g1 = qkv_phase(1)
    ao0 = attn_phase(0, v0, g0)
    v2, g2 = qkv_phase(2)
    ao1 = attn_phase(1, v1, g1)
    cross_rgb, seo0 = o_pair(0, ao0, ao1)
    v3, g3 = qkv_phase(3)
    ao2 = attn_phase(2, v2, g2)
    se_finish(0, seo0)
    v4, g4 = qkv_phase(4)
    ao3 = attn_phase(3, v3, g3)
    cross_dep, seo1 = o_pair(1, ao2, ao3)
    v5, g5 = qkv_phase(5)
    ao4 = attn_phase(4, v4, g4)

    # quality metrics (from the rdl tiles = normalized features + o-bias;
    # the o-bias shift is compensated exactly) - fill the SE wait
    rsum_ps = psum.tile([1, NT], F32, name="rsum_ps", tag="pat", bufs=2)
    for ct in range(8):
        mm(rsum_ps, ones_col_bf, rdl[0][ct], start=(ct == 0), stop=(ct == 7))
    # var = 1/1023 - (rsum' - C)^2/(1024*1023)  with C = sum(bias_o_rgb)
    xr = []
    var_row = sb.tile([1, NT], BF16, name="var_row", tag="row", bufs=4)
    nc.scalar.activation(var_row, rsum_ps, AF.Square,
                         scale=float(1.0 / np.sqrt(1024.0 * 1023.0)),
                         bias=desc[0:1, 21:22])
    nc.vector.tensor_scalar(var_row, var_row, -1.0, 1.0 / 1023.0,
                            op0=ALU.mult, op1=ALU.add)
    xr.append(var_row)
    xr.append(sparsity_metric(1))

    se_finish(1, seo1)
    crosses_rd = [cross_rgb, cross_dep]

    # rgb/depth contributions to the gate conv - filler while AR(5) flies
    gt_ps = psum.tile([3, NT], F32, name="gt_ps", tag="prow", bufs=1)
    for m in range(2):
        for ct in range(8):
            kt = m * 8 + ct
            mm(gt_ps, wg_eff[:, kt * 3:(kt + 1) * 3], crosses_rd[m][ct],
               start=(kt == 0), stop=False)

    ao5 = attn_phase(5, v5, g5)
    cross_lid, seo2 = o_pair(2, ao4, ao5)
    xr.append(sparsity_metric(2))      # fills the SE(2) AllReduce wait
    se_finish(2, seo2)
    crosses = [cross_rgb, cross_dep, cross_lid]

    # lidar part + extras of the gate conv; rgb/depth parts were emitted early
    for ct in range(8):
        mm(gt_ps, wg_eff[:, (16 + ct) * 3:(17 + ct) * 3], crosses[2][ct],
           start=False, stop=False)
    for m in range(3):
        mm(gt_ps, w_gate_x[0:1, 3 * m:3 * m + 3], xr[m],
           start=False, stop=(m == 2))
    gates = sb.tile([3, NT], BF16, name="gates", tag="row", bufs=4)
    nc.scalar.activation(gates, gt_ps, AF.Sigmoid, bias=b_gate[:, 0:1])

    # ---------------- Phase E: fused features + fusion conv ----------------
    # fused = cross * se_gate * modality_gate, in place in one pass
    for m in range(3):
        bc_ps = psum.tile([P, NT], F32, name=f"gbc{m}", tag="pmm", bufs=3)
        mm(bc_ps, sel3[:, m * P:(m + 1) * P], gates)
        for ct in range(8):
            nc.vector.scalar_tensor_tensor(
                crosses[m][ct], crosses[m][ct], gse[m][:, ct:ct + 1], bc_ps,
                op0=ALU.mult, op1=ALU.mult)
    fused = [crosses[m][ct] for m in range(3) for ct in range(8)]

    for ct in range(24):
        wf = sb.tile([P, 24 * P], BF16, name=f"wf{ct}", tag="wfuse", bufs=2)
        eng = nc.sync if ct % 2 == 0 else nc.scalar
        eng.dma_start(wf, p["w_fuse"][:, ct * TOTAL:(ct + 1) * TOTAL])
        ps = psum.tile([P, NT], F32, name=f"ps_f{ct}", tag="pmm", bufs=3)
        for kt in range(24):
            mm(ps, wf[:, kt * P:(kt + 1) * P], fused[kt],
               start=(kt == 0), stop=(kt == 23))
        ot = sb.tile([P, NT], F32, name=f"ot{ct}", tag="outb", bufs=2)
        nc.scalar.activation(ot, ps, AF.Identity, bias=bias_fuse[:, ct: ct + 1])
        nc.gpsimd.dma_start(p["out"][ct * P:(ct + 1) * P, :], ot)


def _build():
    nc = bacc.Bacc("TRN2", target_bir_lowering=False, debug=False,
                   num_devices=NCORES)
    params = _declare_params(nc)
    with tile.TileContext(nc) as tc, ExitStack() as ctx:
        _emit(nc, tc, params, ctx)
    nc.compile()
    return nc


def _fp8_slab(wT, s):
    """[1024, 1024] (already [Cin, Cout]) -> [128, 8192] fp8 DoubleRow image."""
    q8 = (wT * s).astype(e4m3)
    return np.ascontiguousarray(
        q8.reshape(4, 2, P, BC).transpose(2, 0, 1, 3).reshape(P, 8192))


def _pow2_scale(w):
    """largest power of 2 with max|w|*s <= 224"""
    m = float(np.abs(w).max())
    return 2.0 ** np.floor(np.log2(224.0 / m))


def _prep_static(inputs):
    """Host-side weight prep shared by all cores."""
    f32 = np.float32

    def colpack(b):  # [1024] -> [128, 8] (col j = channel tile j)
        return np.ascontiguousarray(b.reshape(8, P).T.astype(f32))

    def packslab(wT, kt):   # [kt*128, n] -> [128, kt*n] SBUF image
        n = wT.shape[1]
        return np.ascontiguousarray(
            wT.reshape(kt, -1, n).transpose(1, 0, 2).reshape(-1, kt * n))

    s = {}
    s["w_rgb"] = packslab(inputs["proj_rgb_w"].T.astype(bf16), 4)
    s["w_depth"] = packslab(inputs["proj_depth_w"].T.astype(bf16), 2)
    s["w_lidar"] = np.ascontiguousarray(inputs["proj_lidar_w"].T.astype(bf16))

    desc = np.zeros((P, 24), f32)
    for nm, base in (("q", 0), ("k", 6), ("v", 12)):
        w = inputs[f"attn_{nm}_w"]
        slabs = np.empty((6, P, 8192), e4m3)
        for i in range(6):
            wT = np.ascontiguousarray(w[i].T)
            sc = _pow2_scale(wT)
            slabs[i] = _fp8_slab(wT, sc)
            desc[:, base + i] = 1.0 / (sc * ASCALE)
        s[f"w8_{nm}"] = slabs
    wo = inputs["attn_o_w"]
    slabs = np.empty((6, P, 8192), e4m3)
    for m in range(3):
        sc = min(_pow2_scale(wo[2 * m].T), _pow2_scale(wo[2 * m + 1].T))
        slabs[2 * m] = _fp8_slab(np.ascontiguousarray(wo[2 * m].T), sc)
        slabs[2 * m + 1] = _fp8_slab(np.ascontiguousarray(wo[2 * m + 1].T), sc)
        desc[:, 18 + m] = 1.0 / (sc * ASCALE)
    s["w8_o"] = slabs
    # var-metric bias: -sum(bias_o_rgb)/sqrt(1024*1023), from the
    # bf16-rounded bias actually folded into the rdl tiles
    bo_rgb = (inputs["attn_o_b"][0] + inputs["attn_o_b"][1]) \
        .astype(bf16).astype(np.float64)
    desc[0, 21] = float(-bo_rgb.sum() / np.sqrt(1024.0 * 1023.0))
    s["desc"] = desc

    qk = np.empty((12, BC), f32)
    for i in range(6):
        qk[2 * i] = inputs["attn_q_b"][i]
        qk[2 * i + 1] = inputs["attn_k_b"][i]
    s["qk_bias"] = np.ascontiguousarray(
        np.broadcast_to(qk[:, None, :], (12, P, BC))).astype(bf16)
    s["bias_proj"] = np.concatenate(
        [colpack(inputs["proj_rgb_b"]), colpack(inputs["proj_depth_b"]),
         colpack(inputs["proj_lidar_b"])], axis=1)
    s["bias_v"] = np.concatenate(
        [colpack(inputs["attn_v_b"][i]) for i in range(6)], axis=1)
    # round the pair-summed o-bias through bf16 so that the on-device
    # is_equal sparsity test (rdl == bias_o) is exact for true zeros
    s["bias_o"] = np.concatenate(
        [colpack((inputs["attn_o_b"][2 * m] + inputs["attn_o_b"][2 * m + 1])
                 .astype(bf16).astype(np.float32))
         for m in range(3)], axis=1)
    s["w_se1"] = np.ascontiguousarray(
        (inputs["se_w1"] / 1024.0).transpose(0, 2, 1)).astype(bf16)
    s["b_se1"] = np.ascontiguousarray(inputs["se_b1"].T.astype(f32))
    s["w_se2"] = np.ascontiguousarray(inputs["se_w2"].transpose(0, 2, 1)).astype(bf16)
    s["b_se2"] = np.concatenate(
        [colpack(inputs["se_b2"][m]) for m in range(3)], axis=1)
    s["w_gate"] = np.ascontiguousarray(inputs["gate_w"][:, :TOTAL].T).astype(bf16)
    s["w_gate_x"] = np.ascontiguousarray(
        inputs["gate_w"][:, TOTAL:].T.astype(f32)).reshape(1, 9).astype(bf16)
    s["b_gate"] = inputs["gate_b"].reshape(3, 1).astype(f32)
    # fusion slab image: [p, ct, kt, n] contiguous per output tile ct
    wfT = inputs["fusion_w"].T.astype(bf16)   # [k, n] = [3072, 3072]
    s["w_fuse"] = np.ascontiguousarray(
        wfT.reshape(24, P, 24, P).transpose(1, 2, 0, 3).reshape(P, 24 * TOTAL))
    s["bias_fuse"] = np.concatenate(
        [colpack(inputs["fusion_b"][ct * BC:(ct + 1) * BC]) for ct in range(3)],
        axis=1)
    sel = np.zeros((3, 3 * P), bf16)
    for m in range(3):
        sel[m, m * P:(m + 1) * P] = 1.0
    s["sel3"] = sel
    return s


def kernel(**inputs):
    global _compiled, LAST_RESULTS
    if _compiled is None:
        _compiled = _build()
    nc = _compiled

    static = _prep_static(inputs)

    def shard(x, c):  # core c: batch c//2, position half c%2, channel-major
        b, hf = c // 2, c % 2
        C = x.shape[1]
        return np.ascontiguousarray(
            x.reshape(B, C, HW)[b, :, hf * NT:(hf + 1) * NT]).astype(bf16)

    in_maps = []
    for c in range(NCORES):
        m = dict(static)
        m["x_rgb"] = shard(inputs["rgb_features"], c)
        m["x_depth"] = shard(inputs["depth_features"], c)
        m["x_lidar"] = shard(inputs["lidar_features"], c)
        in_maps.append(m)

    res = run_bass_kernel_spmd(nc, in_maps, core_ids=list(range(NCORES)),
                               trace=TRACE)
    LAST_RESULTS = res

    full = np.empty((B, TOTAL, HW), np.float32)
    for c in range(NCORES):
        b, hf = c // 2, c % 2
        full[b, :, hf * NT:(hf + 1) * NT] = res.results[c]["out"]
    return full.reshape(B, TOTAL, 32, 32)
